# revision 5
# baseline (speedup 1.0000x reference)
"""DyConv (dynamic convolution) Trainium2 kernel.

Problem: B=16, C=256, O=256, K=4 experts, 3x3 same-conv on 64x64, with
per-sample attention over experts + InstanceNorm2d(affine=False) input norm.

Strategy: data-parallel over batch across 8 cores (2 samples/core).
Each core:
  - loads its 2 samples of x (fp32) + the full expert weight bank (bf16,
    pre-transposed on host to [K, ctile, 128c, 9*256o]).
  - instance-norm stats via bn_stats/bn_aggr (DVE); normalization fused
    into one ACT pass that also casts to bf16 into a zero-padded 66x66
    spatial layout (so conv taps are plain AP offsets).
  - attention MLP in fp32 on PE (tiny matmuls), softmax via Exp on ACT +
    partition-sum matmul + DRAM round-trip broadcast.
  - per-sample weight aggregation on DVE as scalar_tensor_tensor FMA chain.
  - conv: for each (sample, otile, half of rows), accumulate 2 ctile x 9 tap
    bf16 matmuls into a 4-bank PSUM tile (each stationary weight reused for
    4 N=512 matmuls); drain on DVE fused with the aggregated bias; DMA out.

Emission order is tuned so sample 0's conv matmuls start as early as
possible and sample 1's prep hides under sample 0's conv.
"""

import sys

sys.path.insert(0, "/opt/trn_rl_repo")

import numpy as np
import ml_dtypes

import concourse.bacc as bacc
import concourse.tile as tile
from concourse import mybir
from concourse.bass_utils import run_bass_kernel_spmd

F32 = mybir.dt.float32
BF16 = mybir.dt.bfloat16
AF = mybir.ActivationFunctionType
ALU = mybir.AluOpType

N_CORES = 8
S = 2          # samples per core
C = 256        # in channels
O = 256        # out channels
K = 4          # experts
H = W = 64
HP = WP = 66   # padded spatial
NCT = 2        # C tiles of 128
NOT = 2        # O tiles of 128
EPS = 1e-5
TAPS = [(dy, dx) for dy in (-1, 0, 1) for dx in (-1, 0, 1)]


def build_program():
    nc = bacc.Bacc("TRN2", target_bir_lowering=False, debug=False,
                   num_devices=N_CORES)

    x_d = nc.dram_tensor("x", [S, C, H, W], F32, kind="ExternalInput")
    wt_d = nc.dram_tensor("wt", [K, NCT, 128, 9 * O], BF16, kind="ExternalInput")
    bias_d = nc.dram_tensor("bias", [K, O], F32, kind="ExternalInput")
    fc1wT_d = nc.dram_tensor("fc1wT", [NCT, 128, K], F32, kind="ExternalInput")
    fc1b_d = nc.dram_tensor("fc1b", [K, 1], F32, kind="ExternalInput")
    fc2wT_d = nc.dram_tensor("fc2wT", [K, K], F32, kind="ExternalInput")
    fc2b_d = nc.dram_tensor("fc2b", [K, 1], F32, kind="ExternalInput")
    out_d = nc.dram_tensor("out", [S, O, H, W], F32, kind="ExternalOutput")

    xap = x_d.ap()
    outap = out_d.ap()

    with tile.TileContext(nc) as tc:
        with (
            tc.tile_pool(name="singles", bufs=1) as singles,
            tc.tile_pool(name="xraw", bufs=3) as xraw_pool,
            tc.tile_pool(name="xn", bufs=4) as xn_pool,
            tc.tile_pool(name="acc", bufs=2) as acc_pool,
            tc.tile_pool(name="aggw", bufs=4) as aggw_pool,
            tc.tile_pool(name="aggw3", bufs=3) as aggw3_pool,
            tc.tile_pool(name="stats", bufs=4) as stats_pool,
            tc.tile_pool(name="small", bufs=2) as small_pool,
            tc.tile_pool(name="outs", bufs=3) as out_pool,
            tc.tile_pool(name="cpsum", bufs=2, space="PSUM") as cpsum_pool,
            tc.tile_pool(name="dram", bufs=2, space="DRAM") as dram_pool,
        ):
            # ---- constants / small weights ----
            eps_sb = singles.tile([128, 1], F32, tag="eps")
            nc.vector.memset(eps_sb[:], EPS)
            ones_sb = singles.tile([K, 1], F32, tag="ones")
            nc.vector.memset(ones_sb[:], 1.0)

            fc1wT_sb = []
            for ci in range(NCT):
                t = singles.tile([128, K], F32, tag=f"fc1wT{ci}")
                nc.sync.dma_start(out=t[:], in_=fc1wT_d.ap()[ci])
                fc1wT_sb.append(t)
            fc2wT_sb = singles.tile([K, K], F32, tag="fc2wT")
            nc.sync.dma_start(out=fc2wT_sb[:], in_=fc2wT_d.ap())
            fc1b_sb = singles.tile([K, 1], F32, tag="fc1b")
            nc.sync.dma_start(out=fc1b_sb[:], in_=fc1b_d.ap())
            fc2b_sb = singles.tile([K, 1], F32, tag="fc2b")
            nc.sync.dma_start(out=fc2b_sb[:], in_=fc2b_d.ap())
            bias_sb = singles.tile([K, O], F32, tag="biasK")
            nc.sync.dma_start(out=bias_sb[:], in_=bias_d.ap())

            # ---- big loads ----
            # x on the SP (sync) HWDGE ring; expert weights on the ACT ring
            # so they stream in parallel.
            x_raw = [[None] * NCT for _ in range(S)]
            for s in range(S):
                for ci in range(NCT):
                    t = xraw_pool.tile([128, H, W], F32, tag="xraw")
                    nc.sync.dma_start(
                        out=t[:], in_=xap[s, ci * 128:(ci + 1) * 128, :, :])
                    x_raw[s][ci] = t

            wt_sb = [[None] * NCT for _ in range(K)]
            for ci in range(NCT):
                for k in range(K):
                    t = singles.tile([128, 9 * O], BF16, tag=f"wt{k}_{ci}")
                    nc.scalar.dma_start(out=t[:], in_=wt_d.ap()[k, ci])
                    wt_sb[k][ci] = t

            # ---- padded-xn border memsets (tiny, gpsimd) ----
            xn = [[None] * NCT for _ in range(S)]
            for s in range(S):
                for ci in range(NCT):
                    xt = xn_pool.tile([128, HP, WP], BF16, tag="xn")
                    nc.gpsimd.memset(xt[:, 0, :], 0.0)
                    nc.gpsimd.memset(xt[:, HP - 1, :], 0.0)
                    nc.gpsimd.memset(xt[:, 1:HP - 1, 0], 0.0)
                    nc.gpsimd.memset(xt[:, 1:HP - 1, WP - 1], 0.0)
                    xn[s][ci] = xt

            mv = [[None] * NCT for _ in range(S)]
            attn_bc = [None] * S
            aggb_sb = [[None] * NOT for _ in range(S)]
            # aggw[s][ci] is a list of 3 tap-triple tiles [128, 3, O]
            aggw = [[None] * NCT for _ in range(S)]

            def prep_sample(s):
                # instance-norm stats
                for ci in range(NCT):
                    st = stats_pool.tile([128, 8, 6], F32, tag="bnstats")
                    for j in range(8):
                        nc.vector.bn_stats(
                            out=st[:, j, :],
                            in_=x_raw[s][ci][:, 8 * j:8 * (j + 1), :]
                            .rearrange("p a b -> p (a b)"))
                    m = stats_pool.tile([128, 2], F32, tag="mv")
                    nc.vector.bn_aggr(out=m[:], in_=st[:])
                    mv[s][ci] = m

                # attention MLP (fp32, tiny)
                ph = cpsum_pool.tile([K, 1], F32, tag="cps")
                for ci in range(NCT):
                    nc.tensor.matmul(ph[:], fc1wT_sb[ci][:], mv[s][ci][:, 0:1],
                                     start=(ci == 0), stop=(ci == NCT - 1))
                h_sb = small_pool.tile([K, 1], F32, tag="h")
                nc.scalar.activation(h_sb[:], ph[:], AF.Relu, bias=fc1b_sb[:])
                pl = cpsum_pool.tile([K, 1], F32, tag="cps")
                nc.tensor.matmul(pl[:], fc2wT_sb[:], h_sb[:],
                                 start=True, stop=True)
                exp_t = small_pool.tile([K, 1], F32, tag="expt")
                nc.scalar.activation(exp_t[:], pl[:], AF.Exp, bias=fc2b_sb[:])
                psu = cpsum_pool.tile([1, 1], F32, tag="cps")
                nc.tensor.matmul(psu[:], ones_sb[:], exp_t[:],
                                 start=True, stop=True)
                s_sb = small_pool.tile([1, 1], F32, tag="ssb")
                nc.vector.tensor_copy(s_sb[:], psu[:])

                # DRAM round trip to broadcast exp/sum across partitions
                rt = dram_pool.tile([1, 8], F32, tag="rt")
                nc.gpsimd.dma_start(out=rt[0:1, 0:K], in_=exp_t[:])
                nc.gpsimd.dma_start(out=rt[0:1, K:K + 1], in_=s_sb[:])
                exp_bc = small_pool.tile([128, K], F32, tag="expbc")
                nc.gpsimd.dma_start(out=exp_bc[:],
                                    in_=rt[0:1, 0:K].to_broadcast([128, K]))
                s_bc = small_pool.tile([128, 1], F32, tag="sbc")
                nc.gpsimd.dma_start(out=s_bc[:],
                                    in_=rt[0:1, K:K + 1].to_broadcast([128, 1]))
                r_bc = small_pool.tile([128, 1], F32, tag="rbc")
                nc.vector.reciprocal(out=r_bc[:], in_=s_bc[:])
                abc = small_pool.tile([128, K], F32, tag="attnbc")
                nc.vector.tensor_scalar(abc[:], exp_bc[:], r_bc[:, 0:1], None,
                                        ALU.mult)
                attn_bc[s] = abc
                attn_t = small_pool.tile([K, 1], F32, tag="attnt")
                nc.vector.tensor_mul(attn_t[:], exp_t[:], r_bc[0:K, 0:1])

                # aggregated bias agg_b[o] = sum_k attn[k] bias[k, o]
                for oi in range(NOT):
                    pab = cpsum_pool.tile([128, 1], F32, tag="cps")
                    nc.tensor.matmul(pab[:],
                                     bias_sb[:, oi * 128:(oi + 1) * 128],
                                     attn_t[:], start=True, stop=True)
                    ab = singles.tile([128, 1], F32, tag=f"aggb{s}_{oi}")
                    nc.vector.tensor_copy(ab[:], pab[:])
                    aggb_sb[s][oi] = ab

                # normalize + aggregate weights, ctile by ctile
                for ci in range(NCT):
                    sd = stats_pool.tile([128, 1], F32, tag="sd")
                    nc.scalar.activation(sd[:], mv[s][ci][:, 1:2], AF.Sqrt,
                                         bias=eps_sb[:])
                    rs = stats_pool.tile([128, 1], F32, tag="rs")
                    nc.vector.reciprocal(out=rs[:], in_=sd[:])
                    nmrs = stats_pool.tile([128, 1], F32, tag="nmrs")
                    nc.vector.tensor_scalar(nmrs[:], mv[s][ci][:, 0:1],
                                            rs[:, 0:1], -1.0, ALU.mult,
                                            ALU.mult)
                    nc.scalar.activation(xn[s][ci][:, 1:1 + H, 1:1 + W],
                                         x_raw[s][ci][:], AF.Identity,
                                         bias=nmrs[:, 0:1], scale=rs[:, 0:1])

                    # weight aggregation; ci 0 in tap-triple chunks so the
                    # first conv matmuls can start before the whole agg ends
                    triples = []
                    chunks = 3 if ci == 0 else 1
                    for tr in range(chunks):
                        lo = tr * (9 // chunks) * O
                        hi = (tr + 1) * (9 // chunks) * O
                        ac = acc_pool.tile([128, hi - lo], F32, tag="acc")
                        nc.vector.tensor_scalar(
                            ac[:], wt_sb[0][ci][:, lo:hi],
                            attn_bc[s][:, 0:1], None, ALU.mult)
                        for k in (1, 2):
                            nc.vector.scalar_tensor_tensor(
                                ac[:], wt_sb[k][ci][:, lo:hi],
                                attn_bc[s][:, k:k + 1], ac[:],
                                ALU.mult, ALU.add)
                        aw = (aggw3_pool if chunks == 3 else aggw_pool).tile(
                            [128, (9 // chunks), O], BF16,
                            tag="aggw3" if chunks == 3 else "aggw")
                        nc.vector.scalar_tensor_tensor(
                            aw[:].rearrange("p a b -> p (a b)"),
                            wt_sb[3][ci][:, lo:hi],
                            attn_bc[s][:, 3:4], ac[:], ALU.mult, ALU.add)
                        triples.append(aw)
                    aggw[s][ci] = triples

            def lhsT_for(s, ci, t, oi):
                triples = aggw[s][ci]
                if len(triples) == 3:
                    return triples[t // 3][:, t % 3, oi * 128:(oi + 1) * 128]
                return triples[0][:, t, oi * 128:(oi + 1) * 128]

            def conv_otile(s, oi):
                for half in range(2):
                    ps = cpsum_pool.tile([128, 2048], F32, tag="cps")
                    for ci in range(NCT):
                        for t, (dy, dx) in enumerate(TAPS):
                            lhsT = lhsT_for(s, ci, t, oi)
                            first = (ci == 0 and t == 0)
                            last = (ci == NCT - 1 and t == len(TAPS) - 1)
                            for blk in range(4):
                                y0 = half * 32 + blk * 8
                                rhs = xn[s][ci][:, y0 + 1 + dy:y0 + 9 + dy,
                                                1 + dx:1 + dx + W]
                                nc.tensor.matmul(
                                    ps[:, blk * 512:(blk + 1) * 512],
                                    lhsT, rhs, start=first, stop=last)
                    ot = out_pool.tile([128, 2048], F32, tag="ot")
                    nc.vector.tensor_scalar(ot[:], ps[:],
                                            aggb_sb[s][oi][:, 0:1], None,
                                            ALU.add)
                    nc.sync.dma_start(
                        out=outap[s, oi * 128:(oi + 1) * 128,
                                  half * 32:(half + 1) * 32, :],
                        in_=ot[:])

            prep_sample(0)
            conv_otile(0, 0)
            prep_sample(1)
            conv_otile(0, 1)
            conv_otile(1, 0)
            conv_otile(1, 1)

    nc.compile()
    return nc


_CACHED = {}


def _get_program():
    if "nc" not in _CACHED:
        _CACHED["nc"] = build_program()
    return _CACHED["nc"]


def _prep_shared(weight, bias, fc1_w, fc1_b, fc2_w, fc2_b):
    # weight [K, O, C, 3, 3] -> [K, C, 3*3, O] -> [K, NCT, 128, 9*O], bf16
    wt = np.ascontiguousarray(weight.transpose(0, 2, 3, 4, 1)).reshape(
        K, NCT, 128, 9 * O).astype(ml_dtypes.bfloat16)
    fc1wT = np.ascontiguousarray(fc1_w.T).reshape(NCT, 128, K).astype(np.float32)
    fc2wT = np.ascontiguousarray(fc2_w.T).astype(np.float32)
    return {
        "wt": wt,
        "bias": bias.astype(np.float32),
        "fc1wT": fc1wT,
        "fc1b": fc1_b.reshape(K, 1).astype(np.float32),
        "fc2wT": fc2wT,
        "fc2b": fc2_b.reshape(K, 1).astype(np.float32),
    }


def run(x, weight, bias, fc1_w, fc1_b, fc2_w, fc2_b, trace=False,
        trace_kwargs=None):
    nc = _get_program()
    shared = _prep_shared(weight, bias, fc1_w, fc1_b, fc2_w, fc2_b)
    x = np.asarray(x, dtype=np.float32)
    in_maps = []
    for i in range(N_CORES):
        m = dict(shared)
        m["x"] = np.ascontiguousarray(x[i * S:(i + 1) * S])
        in_maps.append(m)
    res = run_bass_kernel_spmd(nc, in_maps, core_ids=list(range(N_CORES)),
                               trace=trace, **(trace_kwargs or {}))
    out = np.concatenate([res.results[i]["out"] for i in range(N_CORES)],
                         axis=0)
    return out, res


def kernel(x, weight, bias, fc1_w, fc1_b, fc2_w, fc2_b):
    out, _ = run(x, weight, bias, fc1_w, fc1_b, fc2_w, fc2_b)
    return out


# revision 6
# speedup vs baseline: 1.0696x; 1.0696x over previous
"""DyConv (dynamic convolution) Trainium2 kernel.

Problem: B=16, C=256, O=256, K=4 experts, 3x3 same-conv on 64x64, with
per-sample attention over experts + InstanceNorm2d(affine=False) input norm.

Strategy: data-parallel over batch across 8 cores (2 samples/core).
Per core:
  - x (fp32) loads on the SP HWDGE ring; expert weight bank (bf16,
    host-pretransposed to [K, ctile, 128c, 9*256o]) on the ACT ring;
    small weights on the gpsimd (SWDGE) ring.
  - instance-norm stats via two ACT passes with accum_out (sum x, sum x^2);
    the activations' main outputs are dumped into the padded-xn interior,
    which the later normalize pass overwrites.  Attention consumes sum(x)
    directly (fc1wT host-scaled by 1/HW), so the MLP never waits on DVE.
  - attention MLP in fp32 on PE (tiny matmuls), softmax via Exp on ACT +
    partition-sum matmul + DRAM round-trip broadcast (gpsimd ring).
  - per-sample weight aggregation on DVE (scalar_tensor_tensor FMA chain);
    sample0/ctile0 chunked in tap-triples so conv can start early.
  - conv: per (sample, otile, quarter of 16 rows) accumulate 2 ctile x 9 tap
    bf16 matmuls into a 2-bank PSUM tile; drain on DVE fused with the
    aggregated bias; store on the SP ring.
  - emission order defers sample1's PE ops behind sample0's convs so the
    PE never waits on sample1's prep.
"""

import sys

sys.path.insert(0, "/opt/trn_rl_repo")

import numpy as np
import ml_dtypes

import concourse.bacc as bacc
import concourse.tile as tile
from concourse import mybir
from concourse.bass_utils import run_bass_kernel_spmd

F32 = mybir.dt.float32
BF16 = mybir.dt.bfloat16
AF = mybir.ActivationFunctionType
ALU = mybir.AluOpType

N_CORES = 8
S = 2          # samples per core
C = 256        # in channels
O = 256        # out channels
K = 4          # experts
H = W = 64
HP = WP = 66   # padded spatial
NCT = 2        # C tiles of 128
NOT = 2        # O tiles of 128
EPS = 1e-5
INV_HW = 1.0 / (H * W)
TAPS = [(dy, dx) for dy in (-1, 0, 1) for dx in (-1, 0, 1)]


def build_program():
    nc = bacc.Bacc("TRN2", target_bir_lowering=False, debug=False,
                   num_devices=N_CORES)

    x_d = nc.dram_tensor("x", [S, C, H, W], F32, kind="ExternalInput")
    wt_d = nc.dram_tensor("wt", [K, NCT, 128, 9 * O], BF16, kind="ExternalInput")
    bias_d = nc.dram_tensor("bias", [K, O], F32, kind="ExternalInput")
    fc1wT_d = nc.dram_tensor("fc1wT", [NCT, 128, K], F32, kind="ExternalInput")
    fc1b_d = nc.dram_tensor("fc1b", [K, 1], F32, kind="ExternalInput")
    fc2wT_d = nc.dram_tensor("fc2wT", [K, K], F32, kind="ExternalInput")
    fc2b_d = nc.dram_tensor("fc2b", [K, 1], F32, kind="ExternalInput")
    out_d = nc.dram_tensor("out", [S, O, H, W], F32, kind="ExternalOutput")

    xap = x_d.ap()
    outap = out_d.ap()

    with tile.TileContext(nc) as tc:
        with (
            tc.tile_pool(name="singles", bufs=1) as singles,
            tc.tile_pool(name="xraw", bufs=4) as xraw_pool,
            tc.tile_pool(name="xn", bufs=4) as xn_pool,
            tc.tile_pool(name="acc", bufs=2) as acc_pool,
            tc.tile_pool(name="aggw", bufs=2) as aggw_pool,
            tc.tile_pool(name="aggw3", bufs=6) as aggw3_pool,
            tc.tile_pool(name="stats", bufs=4) as stats_pool,
            tc.tile_pool(name="small", bufs=2) as small_pool,
            tc.tile_pool(name="outs", bufs=3) as out_pool,
            tc.tile_pool(name="cpsum", bufs=3, space="PSUM") as cpsum_pool,
            tc.tile_pool(name="dram", bufs=2, space="DRAM") as dram_pool,
        ):
            # ---- constants ----
            eps_sb = singles.tile([128, 1], F32, tag="eps")
            nc.vector.memset(eps_sb[:], EPS)
            ones_sb = singles.tile([K, 1], F32, tag="ones")
            nc.vector.memset(ones_sb[:], 1.0)

            # small weights on the gpsimd ring (keep the SP ring free for x)
            fc1wT_sb = []
            for ci in range(NCT):
                t = singles.tile([128, K], F32, tag=f"fc1wT{ci}")
                nc.gpsimd.dma_start(out=t[:], in_=fc1wT_d.ap()[ci])
                fc1wT_sb.append(t)
            fc2wT_sb = singles.tile([K, K], F32, tag="fc2wT")
            nc.gpsimd.dma_start(out=fc2wT_sb[:], in_=fc2wT_d.ap())
            fc1b_sb = singles.tile([K, 1], F32, tag="fc1b")
            nc.gpsimd.dma_start(out=fc1b_sb[:], in_=fc1b_d.ap())
            fc2b_sb = singles.tile([K, 1], F32, tag="fc2b")
            nc.gpsimd.dma_start(out=fc2b_sb[:], in_=fc2b_d.ap())
            bias_sb = singles.tile([K, O], F32, tag="biasK")
            nc.gpsimd.dma_start(out=bias_sb[:], in_=bias_d.ap())

            # ---- big loads ----
            x_raw = [[None] * NCT for _ in range(S)]
            for s in range(S):
                for ci in range(NCT):
                    t = xraw_pool.tile([128, H, W], F32, tag="xraw")
                    nc.sync.dma_start(
                        out=t[:], in_=xap[s, ci * 128:(ci + 1) * 128, :, :])
                    x_raw[s][ci] = t

            wt_sb = [[None] * NCT for _ in range(K)]
            for ci in range(NCT):
                for k in range(K):
                    t = singles.tile([128, 9 * O], BF16, tag=f"wt{k}_{ci}")
                    nc.scalar.dma_start(out=t[:], in_=wt_d.ap()[k, ci])
                    wt_sb[k][ci] = t

            # ---- padded-xn border memsets (tiny, gpsimd) ----
            xn = [[None] * NCT for _ in range(S)]
            for s in range(S):
                for ci in range(NCT):
                    xt = xn_pool.tile([128, HP, WP], BF16, tag="xn")
                    nc.gpsimd.memset(xt[:, 0, :], 0.0)
                    nc.gpsimd.memset(xt[:, HP - 1, :], 0.0)
                    nc.gpsimd.memset(xt[:, 1:HP - 1, 0], 0.0)
                    nc.gpsimd.memset(xt[:, 1:HP - 1, WP - 1], 0.0)
                    xn[s][ci] = xt

            sumx = [[None] * NCT for _ in range(S)]
            sumsq = [[None] * NCT for _ in range(S)]
            attn_t = [None] * S
            attn_bc = [None] * S
            aggb_sb = [[None] * NOT for _ in range(S)]
            aggw = [[None] * NCT for _ in range(S)]

            def stats_sumx(s):
                # ACT pass: accum_out = sum(x); main output dumped into the
                # (later overwritten) xn interior.
                for ci in range(NCT):
                    sx = stats_pool.tile([128, 1], F32, tag="sumx")
                    nc.scalar.activation(xn[s][ci][:, 1:1 + H, 1:1 + W],
                                         x_raw[s][ci][:], AF.Identity,
                                         accum_out=sx[:])
                    sumx[s][ci] = sx

            def stats_sumsq(s):
                for ci in range(NCT):
                    sq = stats_pool.tile([128, 1], F32, tag="sumsq")
                    nc.scalar.activation(xn[s][ci][:, 1:1 + H, 1:1 + W],
                                         x_raw[s][ci][:], AF.Square,
                                         accum_out=sq[:])
                    sumsq[s][ci] = sq

            def attention_mlp(s):
                # fc1wT is host-scaled by 1/HW so sum(x) is the right input
                ph = cpsum_pool.tile([K, 1], F32, tag="cps")
                for ci in range(NCT):
                    nc.tensor.matmul(ph[:], fc1wT_sb[ci][:], sumx[s][ci][:],
                                     start=(ci == 0), stop=(ci == NCT - 1))
                h_sb = small_pool.tile([K, 1], F32, tag="h")
                nc.scalar.activation(h_sb[:], ph[:], AF.Relu, bias=fc1b_sb[:])
                pl = cpsum_pool.tile([K, 1], F32, tag="cps")
                nc.tensor.matmul(pl[:], fc2wT_sb[:], h_sb[:],
                                 start=True, stop=True)
                exp_t = small_pool.tile([K, 1], F32, tag="expt")
                nc.scalar.activation(exp_t[:], pl[:], AF.Exp, bias=fc2b_sb[:])
                psu = cpsum_pool.tile([1, 1], F32, tag="cps")
                nc.tensor.matmul(psu[:], ones_sb[:], exp_t[:],
                                 start=True, stop=True)
                s_sb = small_pool.tile([1, 1], F32, tag="ssb")
                nc.vector.tensor_copy(s_sb[:], psu[:])

                # DRAM round trip to broadcast exp/sum across partitions
                rt = dram_pool.tile([1, 8], F32, tag="rt")
                nc.gpsimd.dma_start(out=rt[0:1, 0:K], in_=exp_t[:])
                nc.gpsimd.dma_start(out=rt[0:1, K:K + 1], in_=s_sb[:])
                exp_bc = small_pool.tile([128, K], F32, tag="expbc")
                nc.gpsimd.dma_start(out=exp_bc[:],
                                    in_=rt[0:1, 0:K].to_broadcast([128, K]))
                s_bc = small_pool.tile([128, 1], F32, tag="sbc")
                nc.gpsimd.dma_start(out=s_bc[:],
                                    in_=rt[0:1, K:K + 1].to_broadcast([128, 1]))
                r_bc = small_pool.tile([128, 1], F32, tag="rbc")
                nc.vector.reciprocal(out=r_bc[:], in_=s_bc[:])
                abc = small_pool.tile([128, K], F32, tag="attnbc")
                nc.vector.tensor_scalar(abc[:], exp_bc[:], r_bc[:, 0:1], None,
                                        ALU.mult)
                attn_bc[s] = abc
                at = small_pool.tile([K, 1], F32, tag="attnt")
                nc.vector.tensor_mul(at[:], exp_t[:], r_bc[0:K, 0:1])
                attn_t[s] = at

            def agg_bias(s):
                for oi in range(NOT):
                    pab = cpsum_pool.tile([128, 1], F32, tag="cps")
                    nc.tensor.matmul(pab[:],
                                     bias_sb[:, oi * 128:(oi + 1) * 128],
                                     attn_t[s][:], start=True, stop=True)
                    ab = singles.tile([128, 1], F32, tag=f"aggb{s}_{oi}")
                    nc.vector.tensor_copy(ab[:], pab[:])
                    aggb_sb[s][oi] = ab

            def normalize(s):
                for ci in range(NCT):
                    mean = stats_pool.tile([128, 1], F32, tag="mean")
                    nc.vector.tensor_scalar(mean[:], sumx[s][ci][:], INV_HW,
                                            None, ALU.mult)
                    m2 = stats_pool.tile([128, 1], F32, tag="m2")
                    nc.vector.tensor_mul(m2[:], mean[:], mean[:])
                    var = stats_pool.tile([128, 1], F32, tag="var")
                    nc.vector.scalar_tensor_tensor(var[:], sumsq[s][ci][:],
                                                   INV_HW, m2[:], ALU.mult,
                                                   ALU.subtract)
                    sd = stats_pool.tile([128, 1], F32, tag="sd")
                    nc.scalar.activation(sd[:], var[:], AF.Sqrt, bias=eps_sb[:])
                    rs = stats_pool.tile([128, 1], F32, tag="rs")
                    nc.vector.reciprocal(out=rs[:], in_=sd[:])
                    nmrs = stats_pool.tile([128, 1], F32, tag="nmrs")
                    nc.vector.tensor_scalar(nmrs[:], mean[:], rs[:, 0:1], -1.0,
                                            ALU.mult, ALU.mult)
                    nc.scalar.activation(xn[s][ci][:, 1:1 + H, 1:1 + W],
                                         x_raw[s][ci][:], AF.Identity,
                                         bias=nmrs[:, 0:1], scale=rs[:, 0:1])

            def agg_weights(s):
                for ci in range(NCT):
                    triples = []
                    chunks = 3 if ci == 0 else 1
                    for tr in range(chunks):
                        lo = tr * (9 // chunks) * O
                        hi = (tr + 1) * (9 // chunks) * O
                        ac = acc_pool.tile([128, hi - lo], F32, tag="acc")
                        nc.vector.tensor_scalar(
                            ac[:], wt_sb[0][ci][:, lo:hi],
                            attn_bc[s][:, 0:1], None, ALU.mult)
                        for k in (1, 2):
                            nc.vector.scalar_tensor_tensor(
                                ac[:], wt_sb[k][ci][:, lo:hi],
                                attn_bc[s][:, k:k + 1], ac[:],
                                ALU.mult, ALU.add)
                        aw = (aggw3_pool if chunks == 3 else aggw_pool).tile(
                            [128, (9 // chunks), O], BF16,
                            tag="aggw3" if chunks == 3 else "aggw")
                        nc.vector.scalar_tensor_tensor(
                            aw[:].rearrange("p a b -> p (a b)"),
                            wt_sb[3][ci][:, lo:hi],
                            attn_bc[s][:, 3:4], ac[:], ALU.mult, ALU.add)
                        triples.append(aw)
                    aggw[s][ci] = triples

            def lhsT_for(s, ci, t, oi):
                triples = aggw[s][ci]
                if len(triples) == 3:
                    return triples[t // 3][:, t % 3, oi * 128:(oi + 1) * 128]
                return triples[0][:, t, oi * 128:(oi + 1) * 128]

            def conv_otile(s, oi):
                for q in range(4):
                    ps = cpsum_pool.tile([128, 1024], F32, tag="cps")
                    for ci in range(NCT):
                        for t, (dy, dx) in enumerate(TAPS):
                            lhsT = lhsT_for(s, ci, t, oi)
                            first = (ci == 0 and t == 0)
                            last = (ci == NCT - 1 and t == len(TAPS) - 1)
                            for blk in range(2):
                                y0 = q * 16 + blk * 8
                                rhs = xn[s][ci][:, y0 + 1 + dy:y0 + 9 + dy,
                                                1 + dx:1 + dx + W]
                                nc.tensor.matmul(
                                    ps[:, blk * 512:(blk + 1) * 512],
                                    lhsT, rhs, start=first, stop=last)
                    ot = out_pool.tile([128, 1024], F32, tag="ot")
                    nc.vector.tensor_scalar(ot[:], ps[:],
                                            aggb_sb[s][oi][:, 0:1], None,
                                            ALU.add)
                    nc.sync.dma_start(
                        out=outap[s, oi * 128:(oi + 1) * 128,
                                  q * 16:(q + 1) * 16, :],
                        in_=ot[:])

            # ---- emission schedule ----
            stats_sumx(0)
            attention_mlp(0)
            agg_bias(0)
            stats_sumsq(0)
            normalize(0)
            agg_weights(0)
            conv_otile(0, 0)
            # sample 1 prep: ACT/DVE/gpsimd work overlaps conv(0,0); its PE
            # matmuls sit after conv(0,0) in the PE queue.
            stats_sumx(1)
            stats_sumsq(1)
            attention_mlp(1)
            normalize(1)
            agg_weights(1)
            conv_otile(0, 1)
            agg_bias(1)
            conv_otile(1, 0)
            conv_otile(1, 1)

    nc.compile()
    return nc


_CACHED = {}


def _get_program():
    if "nc" not in _CACHED:
        _CACHED["nc"] = build_program()
    return _CACHED["nc"]


def _prep_shared(weight, bias, fc1_w, fc1_b, fc2_w, fc2_b):
    # weight [K, O, C, 3, 3] -> [K, C, 3*3, O] -> [K, NCT, 128, 9*O], bf16
    wt = np.ascontiguousarray(weight.transpose(0, 2, 3, 4, 1)).reshape(
        K, NCT, 128, 9 * O).astype(ml_dtypes.bfloat16)
    # attention consumes sum(x) rather than mean(x): fold 1/HW into fc1
    fc1wT = np.ascontiguousarray(fc1_w.T).reshape(NCT, 128, K).astype(
        np.float32) * np.float32(INV_HW)
    fc2wT = np.ascontiguousarray(fc2_w.T).astype(np.float32)
    return {
        "wt": wt,
        "bias": bias.astype(np.float32),
        "fc1wT": fc1wT,
        "fc1b": fc1_b.reshape(K, 1).astype(np.float32),
        "fc2wT": fc2wT,
        "fc2b": fc2_b.reshape(K, 1).astype(np.float32),
    }


def run(x, weight, bias, fc1_w, fc1_b, fc2_w, fc2_b, trace=False,
        trace_kwargs=None):
    nc = _get_program()
    shared = _prep_shared(weight, bias, fc1_w, fc1_b, fc2_w, fc2_b)
    x = np.asarray(x, dtype=np.float32)
    in_maps = []
    for i in range(N_CORES):
        m = dict(shared)
        m["x"] = np.ascontiguousarray(x[i * S:(i + 1) * S])
        in_maps.append(m)
    res = run_bass_kernel_spmd(nc, in_maps, core_ids=list(range(N_CORES)),
                               trace=trace, **(trace_kwargs or {}))
    out = np.concatenate([res.results[i]["out"] for i in range(N_CORES)],
                         axis=0)
    return out, res


def kernel(x, weight, bias, fc1_w, fc1_b, fc2_w, fc2_b):
    out, _ = run(x, weight, bias, fc1_w, fc1_b, fc2_w, fc2_b)
    return out


# revision 8
# speedup vs baseline: 1.1101x; 1.0379x over previous
"""DyConv (dynamic convolution) Trainium2 kernel.

Problem: B=16, C=256, O=256, K=4 experts, 3x3 same-conv on 64x64, with
per-sample attention over experts + InstanceNorm2d(affine=False) input norm.

Strategy: data-parallel over batch across 8 cores (2 samples/core).
Per core:
  - x (fp32) loads on the SP HWDGE ring; expert weight bank (bf16,
    host-pretransposed to [K, ctile, 128c, 9*256o]) on the ACT ring;
    small weights on the gpsimd (SWDGE) ring.
  - instance-norm stats via two ACT passes with accum_out (sum x, sum x^2);
    the activations' main outputs are dumped into the padded-xn interior,
    which the later normalize pass overwrites.  Attention consumes sum(x)
    directly (fc1wT host-scaled by 1/HW), so the MLP never waits on DVE.
  - attention MLP in fp32 on PE (tiny matmuls), softmax via Exp on ACT +
    partition-sum matmul + DRAM round-trip broadcast (gpsimd ring).
  - per-sample weight aggregation on DVE (scalar_tensor_tensor FMA chain);
    sample0/ctile0 chunked in tap-triples so conv can start early.
  - conv: per (sample, otile, quarter of 16 rows) accumulate 2 ctile x 9 tap
    bf16 matmuls into a 2-bank PSUM tile; drain on DVE fused with the
    aggregated bias; store on the SP ring.
  - emission order defers sample1's PE ops behind sample0's convs so the
    PE never waits on sample1's prep.
"""

import sys

sys.path.insert(0, "/opt/trn_rl_repo")

import numpy as np
import ml_dtypes

import concourse.bacc as bacc
import concourse.tile as tile
from concourse import mybir
from concourse.bass_utils import run_bass_kernel_spmd

F32 = mybir.dt.float32
BF16 = mybir.dt.bfloat16
AF = mybir.ActivationFunctionType
ALU = mybir.AluOpType

N_CORES = 8
S = 2          # samples per core
C = 256        # in channels
O = 256        # out channels
K = 4          # experts
H = W = 64
HP = WP = 66   # padded spatial
NCT = 2        # C tiles of 128
NOT = 2        # O tiles of 128
EPS = 1e-5
INV_HW = 1.0 / (H * W)
TAPS = [(dy, dx) for dy in (-1, 0, 1) for dx in (-1, 0, 1)]


def build_program():
    nc = bacc.Bacc("TRN2", target_bir_lowering=False, debug=False,
                   num_devices=N_CORES)

    x_d = nc.dram_tensor("x", [S, C, H, W], F32, kind="ExternalInput")
    wt_d = nc.dram_tensor("wt", [K, NCT, 128, 9 * O], BF16, kind="ExternalInput")
    bias_d = nc.dram_tensor("bias", [K, O], F32, kind="ExternalInput")
    fc1wT_d = nc.dram_tensor("fc1wT", [NCT, 128, K], F32, kind="ExternalInput")
    fc1b_d = nc.dram_tensor("fc1b", [K, 1], F32, kind="ExternalInput")
    fc2wT_d = nc.dram_tensor("fc2wT", [K, K], F32, kind="ExternalInput")
    fc2b_d = nc.dram_tensor("fc2b", [K, 1], F32, kind="ExternalInput")
    e5_d = nc.dram_tensor("e5", [K, K + 1], F32, kind="ExternalInput")
    out_d = nc.dram_tensor("out", [S, O, H, W], F32, kind="ExternalOutput")

    xap = x_d.ap()
    outap = out_d.ap()

    with tile.TileContext(nc) as tc:
        with (
            tc.tile_pool(name="singles", bufs=1) as singles,
            tc.tile_pool(name="xraw", bufs=4) as xraw_pool,
            tc.tile_pool(name="xn", bufs=4) as xn_pool,
            tc.tile_pool(name="acc", bufs=2) as acc_pool,
            tc.tile_pool(name="aggw", bufs=2) as aggw_pool,
            tc.tile_pool(name="aggw3", bufs=6) as aggw3_pool,
            tc.tile_pool(name="stats", bufs=4) as stats_pool,
            tc.tile_pool(name="small", bufs=2) as small_pool,
            tc.tile_pool(name="outs", bufs=3) as out_pool,
            tc.tile_pool(name="cpsum", bufs=3, space="PSUM") as cpsum_pool,
        ):
            # ---- constants ----
            eps_sb = singles.tile([128, 1], F32, tag="eps")
            nc.vector.memset(eps_sb[:], EPS)
            e5_sb = singles.tile([K, K + 1], F32, tag="e5")
            nc.gpsimd.dma_start(out=e5_sb[:], in_=e5_d.ap())
            ones1_sb = singles.tile([1, 128], F32, tag="ones1")
            nc.vector.memset(ones1_sb[:], 1.0)

            # small weights on the gpsimd ring (keep the SP ring free for x)
            fc1wT_sb = []
            for ci in range(NCT):
                t = singles.tile([128, K], F32, tag=f"fc1wT{ci}")
                nc.gpsimd.dma_start(out=t[:], in_=fc1wT_d.ap()[ci])
                fc1wT_sb.append(t)
            fc2wT_sb = singles.tile([K, K], F32, tag="fc2wT")
            nc.gpsimd.dma_start(out=fc2wT_sb[:], in_=fc2wT_d.ap())
            fc1b_sb = singles.tile([K, 1], F32, tag="fc1b")
            nc.gpsimd.dma_start(out=fc1b_sb[:], in_=fc1b_d.ap())
            fc2b_sb = singles.tile([K, 1], F32, tag="fc2b")
            nc.gpsimd.dma_start(out=fc2b_sb[:], in_=fc2b_d.ap())
            bias_sb = singles.tile([K, O], F32, tag="biasK")
            nc.gpsimd.dma_start(out=bias_sb[:], in_=bias_d.ap())

            # ---- big loads: one HWDGE ring, priority order ----
            x_raw = [[None] * NCT for _ in range(S)]
            for ci in range(NCT):
                t = xraw_pool.tile([128, H, W], F32, tag="xraw")
                nc.sync.dma_start(out=t[:], in_=xap[0, ci * 128:(ci + 1) * 128, :, :])
                x_raw[0][ci] = t

            wt_sb = [[None] * NCT for _ in range(K)]
            for ci in range(NCT):
                for k in range(K):
                    t = singles.tile([128, 9 * O], BF16, tag=f"wt{k}_{ci}")
                    nc.sync.dma_start(out=t[:], in_=wt_d.ap()[k, ci])
                    wt_sb[k][ci] = t

            for ci in range(NCT):
                t = xraw_pool.tile([128, H, W], F32, tag="xraw")
                nc.sync.dma_start(out=t[:], in_=xap[1, ci * 128:(ci + 1) * 128, :, :])
                x_raw[1][ci] = t

            # ---- padded-xn border memsets (tiny, gpsimd) ----
            xn = [[None] * NCT for _ in range(S)]
            for s in range(S):
                for ci in range(NCT):
                    xt = xn_pool.tile([128, HP, WP], BF16, tag="xn")
                    nc.gpsimd.memset(xt[:, 0, :], 0.0)
                    nc.gpsimd.memset(xt[:, HP - 1, :], 0.0)
                    nc.gpsimd.memset(xt[:, 1:HP - 1, 0], 0.0)
                    nc.gpsimd.memset(xt[:, 1:HP - 1, WP - 1], 0.0)
                    xn[s][ci] = xt

            sumx = [[None] * NCT for _ in range(S)]
            sumsq = [[None] * NCT for _ in range(S)]
            attn_t = [None] * S
            attn_bc = [None] * S
            aggb_sb = [[None] * NOT for _ in range(S)]
            aggw = [[None] * NCT for _ in range(S)]

            def stats_sumx(s):
                # ACT pass: accum_out = sum(x); main output dumped into the
                # (later overwritten) xn interior.
                for ci in range(NCT):
                    sx = stats_pool.tile([128, 1], F32, tag="sumx")
                    nc.scalar.activation(xn[s][ci][:, 1:1 + H, 1:1 + W],
                                         x_raw[s][ci][:], AF.Identity,
                                         accum_out=sx[:])
                    sumx[s][ci] = sx

            def stats_sumsq(s):
                for ci in range(NCT):
                    sq = stats_pool.tile([128, 1], F32, tag="sumsq")
                    nc.scalar.activation(xn[s][ci][:, 1:1 + H, 1:1 + W],
                                         x_raw[s][ci][:], AF.Square,
                                         accum_out=sq[:])
                    sumsq[s][ci] = sq

            def attention_mlp(s):
                # fc1wT is host-scaled by 1/HW so sum(x) is the right input
                ph = cpsum_pool.tile([K, 1], F32, tag="cps")
                for ci in range(NCT):
                    nc.tensor.matmul(ph[:], fc1wT_sb[ci][:], sumx[s][ci][:],
                                     start=(ci == 0), stop=(ci == NCT - 1))
                h_sb = small_pool.tile([K, 1], F32, tag="h")
                nc.scalar.activation(h_sb[:], ph[:], AF.Relu, bias=fc1b_sb[:])
                pl = cpsum_pool.tile([K, 1], F32, tag="cps")
                nc.tensor.matmul(pl[:], fc2wT_sb[:], h_sb[:],
                                 start=True, stop=True)
                exp_t = small_pool.tile([K, 1], F32, tag="expt")
                nc.scalar.activation(exp_t[:], pl[:], AF.Exp, bias=fc2b_sb[:])
                # on-chip softmax broadcast: exp_t.T @ [eye|ones] gives the
                # exp row + its sum on partition 0; recip the sum; then
                # ones-column matmul broadcasts [e0..e3, 1/s] to all partitions
                p5 = cpsum_pool.tile([1, K + 1], F32, tag="cps")
                nc.tensor.matmul(p5[:], exp_t[:], e5_sb[:],
                                 start=True, stop=True)
                row5 = small_pool.tile([1, K + 1], F32, tag="row5")
                nc.vector.tensor_copy(row5[0:1, 0:K], p5[0:1, 0:K])
                nc.vector.reciprocal(out=row5[0:1, K:K + 1],
                                     in_=p5[0:1, K:K + 1])
                pbc = cpsum_pool.tile([128, K + 1], F32, tag="cps")
                nc.tensor.matmul(pbc[:], ones1_sb[:], row5[:],
                                 start=True, stop=True)
                abc = small_pool.tile([128, K], F32, tag="attnbc")
                nc.vector.tensor_scalar(abc[:], pbc[:, 0:K],
                                        pbc[:, K:K + 1], None, ALU.mult)
                attn_bc[s] = abc
                at = small_pool.tile([K, 1], F32, tag="attnt")
                nc.vector.tensor_mul(at[:], exp_t[:], pbc[0:K, K:K + 1])
                attn_t[s] = at

            def agg_bias(s):
                for oi in range(NOT):
                    pab = cpsum_pool.tile([128, 1], F32, tag="cps")
                    nc.tensor.matmul(pab[:],
                                     bias_sb[:, oi * 128:(oi + 1) * 128],
                                     attn_t[s][:], start=True, stop=True)
                    ab = singles.tile([128, 1], F32, tag=f"aggb{s}_{oi}")
                    nc.vector.tensor_copy(ab[:], pab[:])
                    aggb_sb[s][oi] = ab

            def normalize(s):
                for ci in range(NCT):
                    mean = stats_pool.tile([128, 1], F32, tag="mean")
                    nc.vector.tensor_scalar(mean[:], sumx[s][ci][:], INV_HW,
                                            None, ALU.mult)
                    m2 = stats_pool.tile([128, 1], F32, tag="m2")
                    nc.vector.tensor_mul(m2[:], mean[:], mean[:])
                    var = stats_pool.tile([128, 1], F32, tag="var")
                    nc.vector.scalar_tensor_tensor(var[:], sumsq[s][ci][:],
                                                   INV_HW, m2[:], ALU.mult,
                                                   ALU.subtract)
                    sd = stats_pool.tile([128, 1], F32, tag="sd")
                    nc.scalar.activation(sd[:], var[:], AF.Sqrt, bias=eps_sb[:])
                    rs = stats_pool.tile([128, 1], F32, tag="rs")
                    nc.vector.reciprocal(out=rs[:], in_=sd[:])
                    nmrs = stats_pool.tile([128, 1], F32, tag="nmrs")
                    nc.vector.tensor_scalar(nmrs[:], mean[:], rs[:, 0:1], -1.0,
                                            ALU.mult, ALU.mult)
                    nc.scalar.activation(xn[s][ci][:, 1:1 + H, 1:1 + W],
                                         x_raw[s][ci][:], AF.Identity,
                                         bias=nmrs[:, 0:1], scale=rs[:, 0:1])

            def agg_weights(s):
                for ci in range(NCT):
                    triples = []
                    chunks = 3 if ci == 0 else 1
                    for tr in range(chunks):
                        lo = tr * (9 // chunks) * O
                        hi = (tr + 1) * (9 // chunks) * O
                        ac = acc_pool.tile([128, hi - lo], F32, tag="acc")
                        nc.vector.tensor_scalar(
                            ac[:], wt_sb[0][ci][:, lo:hi],
                            attn_bc[s][:, 0:1], None, ALU.mult)
                        for k in (1, 2):
                            nc.vector.scalar_tensor_tensor(
                                ac[:], wt_sb[k][ci][:, lo:hi],
                                attn_bc[s][:, k:k + 1], ac[:],
                                ALU.mult, ALU.add)
                        aw = (aggw3_pool if chunks == 3 else aggw_pool).tile(
                            [128, (9 // chunks), O], BF16,
                            tag="aggw3" if chunks == 3 else "aggw")
                        nc.vector.scalar_tensor_tensor(
                            aw[:].rearrange("p a b -> p (a b)"),
                            wt_sb[3][ci][:, lo:hi],
                            attn_bc[s][:, 3:4], ac[:], ALU.mult, ALU.add)
                        triples.append(aw)
                    aggw[s][ci] = triples

            def lhsT_for(s, ci, t, oi):
                triples = aggw[s][ci]
                if len(triples) == 3:
                    return triples[t // 3][:, t % 3, oi * 128:(oi + 1) * 128]
                return triples[0][:, t, oi * 128:(oi + 1) * 128]

            def conv_otile(s, oi):
                for q in range(4):
                    ps = cpsum_pool.tile([128, 1024], F32, tag="cps")
                    for ci in range(NCT):
                        for t, (dy, dx) in enumerate(TAPS):
                            lhsT = lhsT_for(s, ci, t, oi)
                            first = (ci == 0 and t == 0)
                            last = (ci == NCT - 1 and t == len(TAPS) - 1)
                            for blk in range(2):
                                y0 = q * 16 + blk * 8
                                rhs = xn[s][ci][:, y0 + 1 + dy:y0 + 9 + dy,
                                                1 + dx:1 + dx + W]
                                nc.tensor.matmul(
                                    ps[:, blk * 512:(blk + 1) * 512],
                                    lhsT, rhs, start=first, stop=last)
                    ot = out_pool.tile([128, 1024], F32, tag="ot")
                    nc.vector.tensor_scalar(ot[:], ps[:],
                                            aggb_sb[s][oi][:, 0:1], None,
                                            ALU.add)
                    nc.sync.dma_start(
                        out=outap[s, oi * 128:(oi + 1) * 128,
                                  q * 16:(q + 1) * 16, :],
                        in_=ot[:])

            # ---- emission schedule ----
            stats_sumx(0)
            attention_mlp(0)
            agg_bias(0)
            stats_sumsq(0)
            normalize(0)
            agg_weights(0)
            conv_otile(0, 0)
            # sample 1 prep: ACT/DVE/gpsimd work overlaps conv(0,0); its PE
            # matmuls sit after conv(0,0) in the PE queue.
            stats_sumx(1)
            stats_sumsq(1)
            attention_mlp(1)
            normalize(1)
            agg_weights(1)
            conv_otile(0, 1)
            agg_bias(1)
            conv_otile(1, 0)
            conv_otile(1, 1)

    nc.compile()
    return nc


_CACHED = {}


def _get_program():
    if "nc" not in _CACHED:
        _CACHED["nc"] = build_program()
    return _CACHED["nc"]


def _prep_shared(weight, bias, fc1_w, fc1_b, fc2_w, fc2_b):
    # weight [K, O, C, 3, 3] -> [K, C, 3*3, O] -> [K, NCT, 128, 9*O], bf16
    wt = np.ascontiguousarray(weight.transpose(0, 2, 3, 4, 1)).reshape(
        K, NCT, 128, 9 * O).astype(ml_dtypes.bfloat16)
    # attention consumes sum(x) rather than mean(x): fold 1/HW into fc1
    fc1wT = np.ascontiguousarray(fc1_w.T).reshape(NCT, 128, K).astype(
        np.float32) * np.float32(INV_HW)
    fc2wT = np.ascontiguousarray(fc2_w.T).astype(np.float32)
    return {
        "wt": wt,
        "bias": bias.astype(np.float32),
        "fc1wT": fc1wT,
        "fc1b": fc1_b.reshape(K, 1).astype(np.float32),
        "fc2wT": fc2wT,
        "fc2b": fc2_b.reshape(K, 1).astype(np.float32),
        "e5": np.concatenate([np.eye(K, dtype=np.float32),
                              np.ones((K, 1), np.float32)], axis=1),
    }


def run(x, weight, bias, fc1_w, fc1_b, fc2_w, fc2_b, trace=False,
        trace_kwargs=None):
    nc = _get_program()
    shared = _prep_shared(weight, bias, fc1_w, fc1_b, fc2_w, fc2_b)
    x = np.asarray(x, dtype=np.float32)
    in_maps = []
    for i in range(N_CORES):
        m = dict(shared)
        m["x"] = np.ascontiguousarray(x[i * S:(i + 1) * S])
        in_maps.append(m)
    res = run_bass_kernel_spmd(nc, in_maps, core_ids=list(range(N_CORES)),
                               trace=trace, **(trace_kwargs or {}))
    out = np.concatenate([res.results[i]["out"] for i in range(N_CORES)],
                         axis=0)
    return out, res


def kernel(x, weight, bias, fc1_w, fc1_b, fc2_w, fc2_b):
    out, _ = run(x, weight, bias, fc1_w, fc1_b, fc2_w, fc2_b)
    return out


# revision 11
# speedup vs baseline: 1.1757x; 1.0590x over previous
"""DyConv (dynamic convolution) Trainium2 kernel.

Problem: B=16, C=256, O=256, K=4 experts, 3x3 same-conv on 64x64, with
per-sample attention over experts + InstanceNorm2d(affine=False) input norm.

Strategy: data-parallel over batch across 8 cores (2 samples/core).
Per core:
  - x (fp32) loads on the SP HWDGE ring; expert weight bank (bf16,
    host-pretransposed to [K, ctile, 128c, 9*256o]) on the ACT ring;
    small weights on the gpsimd (SWDGE) ring.
  - instance-norm stats via two ACT passes with accum_out (sum x, sum x^2);
    the activations' main outputs are dumped into the padded-xn interior,
    which the later normalize pass overwrites.  Attention consumes sum(x)
    directly (fc1wT host-scaled by 1/HW), so the MLP never waits on DVE.
  - attention MLP in fp32 on PE (tiny matmuls), softmax via Exp on ACT +
    partition-sum matmul + DRAM round-trip broadcast (gpsimd ring).
  - per-sample weight aggregation on DVE (scalar_tensor_tensor FMA chain);
    sample0/ctile0 chunked in tap-triples so conv can start early.
  - conv: per (sample, otile, quarter of 16 rows) accumulate 2 ctile x 9 tap
    bf16 matmuls into a 2-bank PSUM tile; drain on DVE fused with the
    aggregated bias; store on the SP ring.
  - emission order defers sample1's PE ops behind sample0's convs so the
    PE never waits on sample1's prep.
"""

import sys

sys.path.insert(0, "/opt/trn_rl_repo")

import numpy as np
import ml_dtypes

import concourse.bacc as bacc
import concourse.tile as tile
from concourse import mybir
from concourse.bass_utils import run_bass_kernel_spmd

F32 = mybir.dt.float32
BF16 = mybir.dt.bfloat16
AF = mybir.ActivationFunctionType
ALU = mybir.AluOpType

N_CORES = 8
S = 2          # samples per core
C = 256        # in channels
O = 256        # out channels
K = 4          # experts
H = W = 64
HP = WP = 66   # padded spatial
NCT = 2        # C tiles of 128
NOT = 2        # O tiles of 128
EPS = 1e-5
INV_HW = 1.0 / (H * W)
TAPS = [(dy, dx) for dy in (-1, 0, 1) for dx in (-1, 0, 1)]


def build_program():
    nc = bacc.Bacc("TRN2", target_bir_lowering=False, debug=False,
                   num_devices=N_CORES)

    x_d = nc.dram_tensor("x", [S, C, H, W], F32, kind="ExternalInput")
    wt_d = nc.dram_tensor("wt", [K, NCT, 128, 9 * O], BF16, kind="ExternalInput")
    bias_d = nc.dram_tensor("bias", [K, O], F32, kind="ExternalInput")
    fc1wT_d = nc.dram_tensor("fc1wT", [NCT, 128, K], F32, kind="ExternalInput")
    fc1b_d = nc.dram_tensor("fc1b", [K, 1], F32, kind="ExternalInput")
    fc2wT_d = nc.dram_tensor("fc2wT", [K, K], F32, kind="ExternalInput")
    fc2b_d = nc.dram_tensor("fc2b", [K, 1], F32, kind="ExternalInput")
    e5_d = nc.dram_tensor("e5", [K, K + 1], F32, kind="ExternalInput")
    out_d = nc.dram_tensor("out", [S, O, H, W], F32, kind="ExternalOutput")

    xap = x_d.ap()
    outap = out_d.ap()

    with tile.TileContext(nc) as tc:
        with (
            tc.tile_pool(name="singles", bufs=1) as singles,
            tc.tile_pool(name="xraw", bufs=4) as xraw_pool,
            tc.tile_pool(name="xn", bufs=4) as xn_pool,
            tc.tile_pool(name="acc", bufs=2) as acc_pool,
            tc.tile_pool(name="aggw", bufs=2) as aggw_pool,
            tc.tile_pool(name="aggw3", bufs=6) as aggw3_pool,
            tc.tile_pool(name="stats", bufs=4) as stats_pool,
            tc.tile_pool(name="small", bufs=2) as small_pool,
            tc.tile_pool(name="outs", bufs=3) as out_pool,
            tc.tile_pool(name="cpsum", bufs=3, space="PSUM") as cpsum_pool,
        ):
            # ---- constants ----
            eps_sb = singles.tile([128, 1], F32, tag="eps")
            nc.vector.memset(eps_sb[:], EPS)
            e5_sb = singles.tile([K, K + 1], F32, tag="e5")
            nc.gpsimd.dma_start(out=e5_sb[:], in_=e5_d.ap())
            ones1_sb = singles.tile([1, 128], F32, tag="ones1")
            nc.vector.memset(ones1_sb[:], 1.0)

            # small weights on the gpsimd ring (keep the SP ring free for x)
            fc1wT_sb = []
            for ci in range(NCT):
                t = singles.tile([128, K], F32, tag=f"fc1wT{ci}")
                nc.gpsimd.dma_start(out=t[:], in_=fc1wT_d.ap()[ci])
                fc1wT_sb.append(t)
            fc2wT_sb = singles.tile([K, K], F32, tag="fc2wT")
            nc.gpsimd.dma_start(out=fc2wT_sb[:], in_=fc2wT_d.ap())
            fc1b_sb = singles.tile([K, 1], F32, tag="fc1b")
            nc.gpsimd.dma_start(out=fc1b_sb[:], in_=fc1b_d.ap())
            fc2b_sb = singles.tile([K, 1], F32, tag="fc2b")
            nc.gpsimd.dma_start(out=fc2b_sb[:], in_=fc2b_d.ap())
            bias_sb = singles.tile([K, O], F32, tag="biasK")
            nc.gpsimd.dma_start(out=bias_sb[:], in_=bias_d.ap())

            # ---- big loads: one HWDGE ring, priority order ----
            x_raw = [[None] * NCT for _ in range(S)]
            for ci in range(NCT):
                t = xraw_pool.tile([128, H, W], F32, tag="xraw")
                nc.sync.dma_start(out=t[:], in_=xap[0, ci * 128:(ci + 1) * 128, :, :])
                x_raw[0][ci] = t

            wt_sb = [[None] * NCT for _ in range(K)]
            for ci in range(NCT):
                for k in range(K):
                    t = singles.tile([128, 9 * O], BF16, tag=f"wt{k}_{ci}")
                    nc.sync.dma_start(out=t[:], in_=wt_d.ap()[k, ci])
                    wt_sb[k][ci] = t

            for ci in range(NCT):
                t = xraw_pool.tile([128, H, W], F32, tag="xraw")
                nc.sync.dma_start(out=t[:], in_=xap[1, ci * 128:(ci + 1) * 128, :, :])
                x_raw[1][ci] = t

            # ---- padded-xn border memsets (tiny, gpsimd) ----
            xn = [[None] * NCT for _ in range(S)]
            for s in range(S):
                for ci in range(NCT):
                    xt = xn_pool.tile([128, HP, WP], BF16, tag="xn")
                    nc.gpsimd.memset(xt[:, 0, :], 0.0)
                    nc.gpsimd.memset(xt[:, HP - 1, :], 0.0)
                    nc.gpsimd.memset(xt[:, 1:HP - 1, 0], 0.0)
                    nc.gpsimd.memset(xt[:, 1:HP - 1, WP - 1], 0.0)
                    xn[s][ci] = xt

            mv = [[None] * NCT for _ in range(S)]
            attn_t = [None] * S
            attn_bc = [None] * S
            aggb_sb = [[None] * NOT for _ in range(S)]
            aggw = [[None] * NCT for _ in range(S)]

            def stats(s):
                # instance-norm stats on DVE (idle early); mean feeds attention
                for ci in range(NCT):
                    st = stats_pool.tile([128, 8, 6], F32, tag="bnstats")
                    for j in range(8):
                        nc.vector.bn_stats(
                            out=st[:, j, :],
                            in_=x_raw[s][ci][:, 8 * j:8 * (j + 1), :]
                            .rearrange("p a b -> p (a b)"))
                    m = stats_pool.tile([128, 2], F32, tag="mv")
                    nc.vector.bn_aggr(out=m[:], in_=st[:])
                    mv[s][ci] = m

            def attention_mlp(s):
                ph = cpsum_pool.tile([K, 1], F32, tag="cps")
                for ci in range(NCT):
                    nc.tensor.matmul(ph[:], fc1wT_sb[ci][:], mv[s][ci][:, 0:1],
                                     start=(ci == 0), stop=(ci == NCT - 1))
                h_sb = small_pool.tile([K, 1], F32, tag="h")
                nc.scalar.activation(h_sb[:], ph[:], AF.Relu, bias=fc1b_sb[:])
                pl = cpsum_pool.tile([K, 1], F32, tag="cps")
                nc.tensor.matmul(pl[:], fc2wT_sb[:], h_sb[:],
                                 start=True, stop=True)
                exp_t = small_pool.tile([K, 1], F32, tag="expt")
                nc.scalar.activation(exp_t[:], pl[:], AF.Exp, bias=fc2b_sb[:])
                # on-chip softmax broadcast: exp_t.T @ [eye|ones] gives the
                # exp row + its sum on partition 0; recip the sum; then
                # ones-column matmul broadcasts [e0..e3, 1/s] to all partitions
                p5 = cpsum_pool.tile([1, K + 1], F32, tag="cps")
                nc.tensor.matmul(p5[:], exp_t[:], e5_sb[:],
                                 start=True, stop=True)
                row5 = small_pool.tile([1, K + 1], F32, tag="row5")
                nc.vector.tensor_copy(row5[0:1, 0:K], p5[0:1, 0:K])
                nc.vector.reciprocal(out=row5[0:1, K:K + 1],
                                     in_=p5[0:1, K:K + 1])
                pbc = cpsum_pool.tile([128, K + 1], F32, tag="cps")
                nc.tensor.matmul(pbc[:], ones1_sb[:], row5[:],
                                 start=True, stop=True)
                abc = small_pool.tile([128, K], F32, tag="attnbc")
                nc.vector.tensor_scalar(abc[:], pbc[:, 0:K],
                                        pbc[:, K:K + 1], None, ALU.mult)
                attn_bc[s] = abc
                at = small_pool.tile([K, 1], F32, tag="attnt")
                nc.vector.tensor_mul(at[:], exp_t[:], pbc[0:K, K:K + 1])
                attn_t[s] = at

            def agg_bias(s):
                for oi in range(NOT):
                    pab = cpsum_pool.tile([128, 1], F32, tag="cps")
                    nc.tensor.matmul(pab[:],
                                     bias_sb[:, oi * 128:(oi + 1) * 128],
                                     attn_t[s][:], start=True, stop=True)
                    ab = singles.tile([128, 1], F32, tag=f"aggb{s}_{oi}")
                    nc.vector.tensor_copy(ab[:], pab[:])
                    aggb_sb[s][oi] = ab

            def normalize(s):
                for ci in range(NCT):
                    sd = stats_pool.tile([128, 1], F32, tag="sd")
                    nc.scalar.activation(sd[:], mv[s][ci][:, 1:2], AF.Sqrt,
                                         bias=eps_sb[:])
                    rs = stats_pool.tile([128, 1], F32, tag="rs")
                    nc.vector.reciprocal(out=rs[:], in_=sd[:])
                    nmrs = stats_pool.tile([128, 1], F32, tag="nmrs")
                    nc.vector.tensor_scalar(nmrs[:], mv[s][ci][:, 0:1],
                                            rs[:, 0:1], -1.0, ALU.mult,
                                            ALU.mult)
                    nc.scalar.activation(xn[s][ci][:, 1:1 + H, 1:1 + W],
                                         x_raw[s][ci][:], AF.Identity,
                                         bias=nmrs[:, 0:1], scale=rs[:, 0:1])

            def agg_weights(s):
                for ci in range(NCT):
                    triples = []
                    chunks = 3 if ci == 0 else 1
                    for tr in range(chunks):
                        lo = tr * (9 // chunks) * O
                        hi = (tr + 1) * (9 // chunks) * O
                        ac = acc_pool.tile([128, hi - lo], F32, tag="acc")
                        nc.vector.tensor_scalar(
                            ac[:], wt_sb[0][ci][:, lo:hi],
                            attn_bc[s][:, 0:1], None, ALU.mult)
                        for k in (1, 2):
                            nc.vector.scalar_tensor_tensor(
                                ac[:], wt_sb[k][ci][:, lo:hi],
                                attn_bc[s][:, k:k + 1], ac[:],
                                ALU.mult, ALU.add)
                        aw = (aggw3_pool if chunks == 3 else aggw_pool).tile(
                            [128, (9 // chunks), O], BF16,
                            tag="aggw3" if chunks == 3 else "aggw")
                        nc.vector.scalar_tensor_tensor(
                            aw[:].rearrange("p a b -> p (a b)"),
                            wt_sb[3][ci][:, lo:hi],
                            attn_bc[s][:, 3:4], ac[:], ALU.mult, ALU.add)
                        triples.append(aw)
                    aggw[s][ci] = triples

            def warm_pe():
                # keep the PE busy (HAM warm) while the aggregation finishes;
                # results go to a scratch psum bank and are never read.
                g = x_raw[0][0][:].rearrange("p a b -> p (a b)").bitcast(BF16)
                wp = cpsum_pool.tile([128, 512], F32, tag="warm", bufs=1)
                for i in range(16):
                    nc.tensor.matmul(wp[:], g[:, 0:128], g[:, 512:1024],
                                     start=True, stop=True)

            def lhsT_for(s, ci, t, oi):
                triples = aggw[s][ci]
                if len(triples) == 3:
                    return triples[t // 3][:, t % 3, oi * 128:(oi + 1) * 128]
                return triples[0][:, t, oi * 128:(oi + 1) * 128]

            def conv_otile(s, oi):
                for q in range(4):
                    ps = cpsum_pool.tile([128, 1024], F32, tag="cps")
                    for ci in range(NCT):
                        for t, (dy, dx) in enumerate(TAPS):
                            lhsT = lhsT_for(s, ci, t, oi)
                            first = (ci == 0 and t == 0)
                            last = (ci == NCT - 1 and t == len(TAPS) - 1)
                            for blk in range(2):
                                y0 = q * 16 + blk * 8
                                rhs = xn[s][ci][:, y0 + 1 + dy:y0 + 9 + dy,
                                                1 + dx:1 + dx + W]
                                nc.tensor.matmul(
                                    ps[:, blk * 512:(blk + 1) * 512],
                                    lhsT, rhs, start=first, stop=last)
                    ot = out_pool.tile([128, 1024], F32, tag="ot")
                    nc.vector.tensor_scalar(ot[:], ps[:],
                                            aggb_sb[s][oi][:, 0:1], None,
                                            ALU.add)
                    nc.sync.dma_start(
                        out=outap[s, oi * 128:(oi + 1) * 128,
                                  q * 16:(q + 1) * 16, :],
                        in_=ot[:])

            # ---- emission schedule ----
            stats(0)
            attention_mlp(0)
            agg_bias(0)
            normalize(0)
            agg_weights(0)
            warm_pe()
            stats(1)
            conv_otile(0, 0)
            # rest of sample 1 prep overlaps conv(0,0); its PE matmuls sit
            # after conv(0,0) in the PE queue.
            attention_mlp(1)
            normalize(1)
            agg_weights(1)
            conv_otile(0, 1)
            agg_bias(1)
            conv_otile(1, 0)
            conv_otile(1, 1)

    nc.compile()
    return nc


_CACHED = {}


def _get_program():
    if "nc" not in _CACHED:
        _CACHED["nc"] = build_program()
    return _CACHED["nc"]


def _prep_shared(weight, bias, fc1_w, fc1_b, fc2_w, fc2_b):
    # weight [K, O, C, 3, 3] -> [K, C, 3*3, O] -> [K, NCT, 128, 9*O], bf16
    wt = np.ascontiguousarray(weight.transpose(0, 2, 3, 4, 1)).reshape(
        K, NCT, 128, 9 * O).astype(ml_dtypes.bfloat16)
    fc1wT = np.ascontiguousarray(fc1_w.T).reshape(NCT, 128, K).astype(
        np.float32)
    fc2wT = np.ascontiguousarray(fc2_w.T).astype(np.float32)
    return {
        "wt": wt,
        "bias": bias.astype(np.float32),
        "fc1wT": fc1wT,
        "fc1b": fc1_b.reshape(K, 1).astype(np.float32),
        "fc2wT": fc2wT,
        "fc2b": fc2_b.reshape(K, 1).astype(np.float32),
        "e5": np.concatenate([np.eye(K, dtype=np.float32),
                              np.ones((K, 1), np.float32)], axis=1),
    }


def run(x, weight, bias, fc1_w, fc1_b, fc2_w, fc2_b, trace=False,
        trace_kwargs=None):
    nc = _get_program()
    shared = _prep_shared(weight, bias, fc1_w, fc1_b, fc2_w, fc2_b)
    x = np.asarray(x, dtype=np.float32)
    in_maps = []
    for i in range(N_CORES):
        m = dict(shared)
        m["x"] = np.ascontiguousarray(x[i * S:(i + 1) * S])
        in_maps.append(m)
    res = run_bass_kernel_spmd(nc, in_maps, core_ids=list(range(N_CORES)),
                               trace=trace, **(trace_kwargs or {}))
    out = np.concatenate([res.results[i]["out"] for i in range(N_CORES)],
                         axis=0)
    return out, res


def kernel(x, weight, bias, fc1_w, fc1_b, fc2_w, fc2_b):
    out, _ = run(x, weight, bias, fc1_w, fc1_b, fc2_w, fc2_b)
    return out


# revision 12
# speedup vs baseline: 1.1839x; 1.0070x over previous
"""DyConv (dynamic convolution) Trainium2 kernel.

Problem: B=16, C=256, O=256, K=4 experts, 3x3 same-conv on 64x64, with
per-sample attention over experts + InstanceNorm2d(affine=False) input norm.

Strategy: data-parallel over batch across 8 cores (2 samples/core).
Per core:
  - x (fp32) loads on the SP HWDGE ring; expert weight bank (bf16,
    host-pretransposed to [K, ctile, 128c, 9*256o]) on the ACT ring;
    small weights on the gpsimd (SWDGE) ring.
  - instance-norm stats via two ACT passes with accum_out (sum x, sum x^2);
    the activations' main outputs are dumped into the padded-xn interior,
    which the later normalize pass overwrites.  Attention consumes sum(x)
    directly (fc1wT host-scaled by 1/HW), so the MLP never waits on DVE.
  - attention MLP in fp32 on PE (tiny matmuls), softmax via Exp on ACT +
    partition-sum matmul + DRAM round-trip broadcast (gpsimd ring).
  - per-sample weight aggregation on DVE (scalar_tensor_tensor FMA chain);
    sample0/ctile0 chunked in tap-triples so conv can start early.
  - conv: per (sample, otile, quarter of 16 rows) accumulate 2 ctile x 9 tap
    bf16 matmuls into a 2-bank PSUM tile; drain on DVE fused with the
    aggregated bias; store on the SP ring.
  - emission order defers sample1's PE ops behind sample0's convs so the
    PE never waits on sample1's prep.
"""

import sys

sys.path.insert(0, "/opt/trn_rl_repo")

import numpy as np
import ml_dtypes

import concourse.bacc as bacc
import concourse.tile as tile
from concourse import mybir
from concourse.bass_utils import run_bass_kernel_spmd

F32 = mybir.dt.float32
BF16 = mybir.dt.bfloat16
AF = mybir.ActivationFunctionType
ALU = mybir.AluOpType

N_CORES = 8
S = 2          # samples per core
C = 256        # in channels
O = 256        # out channels
K = 4          # experts
H = W = 64
HP = WP = 66   # padded spatial
NCT = 2        # C tiles of 128
NOT = 2        # O tiles of 128
EPS = 1e-5
INV_HW = 1.0 / (H * W)
TAPS = [(dy, dx) for dy in (-1, 0, 1) for dx in (-1, 0, 1)]


def build_program():
    nc = bacc.Bacc("TRN2", target_bir_lowering=False, debug=False,
                   num_devices=N_CORES)

    x_d = nc.dram_tensor("x", [S, C, H, W], F32, kind="ExternalInput")
    wt_d = nc.dram_tensor("wt", [K, NCT, 128, 9 * O], BF16, kind="ExternalInput")
    bias_d = nc.dram_tensor("bias", [K, O], F32, kind="ExternalInput")
    fc1wT_d = nc.dram_tensor("fc1wT", [NCT, 128, K], F32, kind="ExternalInput")
    fc1b_d = nc.dram_tensor("fc1b", [K, 1], F32, kind="ExternalInput")
    fc2wT_d = nc.dram_tensor("fc2wT", [K, K], F32, kind="ExternalInput")
    fc2b_d = nc.dram_tensor("fc2b", [K, 1], F32, kind="ExternalInput")
    e5_d = nc.dram_tensor("e5", [K, K + 1], F32, kind="ExternalInput")
    out_d = nc.dram_tensor("out", [S, O, H, W], F32, kind="ExternalOutput")

    xap = x_d.ap()
    outap = out_d.ap()

    with tile.TileContext(nc) as tc:
        with (
            tc.tile_pool(name="singles", bufs=1) as singles,
            tc.tile_pool(name="xraw", bufs=4) as xraw_pool,
            tc.tile_pool(name="xn", bufs=4) as xn_pool,
            tc.tile_pool(name="acc", bufs=2) as acc_pool,
            tc.tile_pool(name="aggw", bufs=2) as aggw_pool,
            tc.tile_pool(name="aggw3", bufs=6) as aggw3_pool,
            tc.tile_pool(name="stats", bufs=4) as stats_pool,
            tc.tile_pool(name="small", bufs=2) as small_pool,
            tc.tile_pool(name="outs", bufs=3) as out_pool,
            tc.tile_pool(name="cpsum", bufs=3, space="PSUM") as cpsum_pool,
        ):
            # ---- constants ----
            eps_sb = singles.tile([128, 1], F32, tag="eps")
            nc.vector.memset(eps_sb[:], EPS)
            e5_sb = singles.tile([K, K + 1], F32, tag="e5")
            nc.gpsimd.dma_start(out=e5_sb[:], in_=e5_d.ap())
            ones1_sb = singles.tile([1, 128], F32, tag="ones1")
            nc.vector.memset(ones1_sb[:], 1.0)

            # small weights on the gpsimd ring (keep the SP ring free for x)
            fc1wT_sb = []
            for ci in range(NCT):
                t = singles.tile([128, K], F32, tag=f"fc1wT{ci}")
                nc.gpsimd.dma_start(out=t[:], in_=fc1wT_d.ap()[ci])
                fc1wT_sb.append(t)
            fc2wT_sb = singles.tile([K, K], F32, tag="fc2wT")
            nc.gpsimd.dma_start(out=fc2wT_sb[:], in_=fc2wT_d.ap())
            fc1b_sb = singles.tile([K, 1], F32, tag="fc1b")
            nc.gpsimd.dma_start(out=fc1b_sb[:], in_=fc1b_d.ap())
            fc2b_sb = singles.tile([K, 1], F32, tag="fc2b")
            nc.gpsimd.dma_start(out=fc2b_sb[:], in_=fc2b_d.ap())
            bias_sb = singles.tile([K, O], F32, tag="biasK")
            nc.gpsimd.dma_start(out=bias_sb[:], in_=bias_d.ap())

            # ---- big loads: one HWDGE ring, priority order ----
            x_raw = [[None] * NCT for _ in range(S)]
            for ci in range(NCT):
                t = xraw_pool.tile([128, H, W], F32, tag="xraw")
                for hh in range(2):
                    nc.sync.dma_start(
                        out=t[:, hh * 32:(hh + 1) * 32, :],
                        in_=xap[0, ci * 128:(ci + 1) * 128,
                                hh * 32:(hh + 1) * 32, :])
                x_raw[0][ci] = t

            wt_sb = [[None] * NCT for _ in range(K)]
            for ci in range(NCT):
                for k in range(K):
                    t = singles.tile([128, 9 * O], BF16, tag=f"wt{k}_{ci}")
                    nc.sync.dma_start(out=t[:], in_=wt_d.ap()[k, ci])
                    wt_sb[k][ci] = t

            for ci in range(NCT):
                t = xraw_pool.tile([128, H, W], F32, tag="xraw")
                nc.sync.dma_start(out=t[:], in_=xap[1, ci * 128:(ci + 1) * 128, :, :])
                x_raw[1][ci] = t

            # ---- padded-xn border memsets (tiny, gpsimd) ----
            xn = [[None] * NCT for _ in range(S)]
            for s in range(S):
                for ci in range(NCT):
                    xt = xn_pool.tile([128, HP, WP], BF16, tag="xn")
                    nc.gpsimd.memset(xt[:, 0, :], 0.0)
                    nc.gpsimd.memset(xt[:, HP - 1, :], 0.0)
                    nc.gpsimd.memset(xt[:, 1:HP - 1, 0], 0.0)
                    nc.gpsimd.memset(xt[:, 1:HP - 1, WP - 1], 0.0)
                    xn[s][ci] = xt

            mv = [[None] * NCT for _ in range(S)]
            attn_t = [None] * S
            attn_bc = [None] * S
            aggb_sb = [[None] * NOT for _ in range(S)]
            aggw = [[None] * NCT for _ in range(S)]

            def stats(s):
                # instance-norm stats on DVE (idle early); mean feeds attention
                for ci in range(NCT):
                    st = stats_pool.tile([128, 8, 6], F32, tag="bnstats")
                    for j in range(8):
                        nc.vector.bn_stats(
                            out=st[:, j, :],
                            in_=x_raw[s][ci][:, 8 * j:8 * (j + 1), :]
                            .rearrange("p a b -> p (a b)"))
                    m = stats_pool.tile([128, 2], F32, tag="mv")
                    nc.vector.bn_aggr(out=m[:], in_=st[:])
                    mv[s][ci] = m

            def attention_mlp(s):
                ph = cpsum_pool.tile([K, 1], F32, tag="cps")
                for ci in range(NCT):
                    nc.tensor.matmul(ph[:], fc1wT_sb[ci][:], mv[s][ci][:, 0:1],
                                     start=(ci == 0), stop=(ci == NCT - 1))
                h_sb = small_pool.tile([K, 1], F32, tag="h")
                nc.scalar.activation(h_sb[:], ph[:], AF.Relu, bias=fc1b_sb[:])
                pl = cpsum_pool.tile([K, 1], F32, tag="cps")
                nc.tensor.matmul(pl[:], fc2wT_sb[:], h_sb[:],
                                 start=True, stop=True)
                exp_t = small_pool.tile([K, 1], F32, tag="expt")
                nc.scalar.activation(exp_t[:], pl[:], AF.Exp, bias=fc2b_sb[:])
                # on-chip softmax broadcast: exp_t.T @ [eye|ones] gives the
                # exp row + its sum on partition 0; recip the sum; then
                # ones-column matmul broadcasts [e0..e3, 1/s] to all partitions
                p5 = cpsum_pool.tile([1, K + 1], F32, tag="cps")
                nc.tensor.matmul(p5[:], exp_t[:], e5_sb[:],
                                 start=True, stop=True)
                row5 = small_pool.tile([1, K + 1], F32, tag="row5")
                nc.vector.tensor_copy(row5[0:1, 0:K], p5[0:1, 0:K])
                nc.vector.reciprocal(out=row5[0:1, K:K + 1],
                                     in_=p5[0:1, K:K + 1])
                pbc = cpsum_pool.tile([128, K + 1], F32, tag="cps")
                nc.tensor.matmul(pbc[:], ones1_sb[:], row5[:],
                                 start=True, stop=True)
                abc = small_pool.tile([128, K], F32, tag="attnbc")
                nc.vector.tensor_scalar(abc[:], pbc[:, 0:K],
                                        pbc[:, K:K + 1], None, ALU.mult)
                attn_bc[s] = abc
                at = small_pool.tile([K, 1], F32, tag="attnt")
                nc.vector.tensor_mul(at[:], exp_t[:], pbc[0:K, K:K + 1])
                attn_t[s] = at

            def agg_bias(s):
                for oi in range(NOT):
                    pab = cpsum_pool.tile([128, 1], F32, tag="cps")
                    nc.tensor.matmul(pab[:],
                                     bias_sb[:, oi * 128:(oi + 1) * 128],
                                     attn_t[s][:], start=True, stop=True)
                    ab = singles.tile([128, 1], F32, tag=f"aggb{s}_{oi}")
                    nc.vector.tensor_copy(ab[:], pab[:])
                    aggb_sb[s][oi] = ab

            def normalize(s):
                for ci in range(NCT):
                    sd = stats_pool.tile([128, 1], F32, tag="sd")
                    nc.scalar.activation(sd[:], mv[s][ci][:, 1:2], AF.Sqrt,
                                         bias=eps_sb[:])
                    rs = stats_pool.tile([128, 1], F32, tag="rs")
                    nc.vector.reciprocal(out=rs[:], in_=sd[:])
                    nmrs = stats_pool.tile([128, 1], F32, tag="nmrs")
                    nc.vector.tensor_scalar(nmrs[:], mv[s][ci][:, 0:1],
                                            rs[:, 0:1], -1.0, ALU.mult,
                                            ALU.mult)
                    nc.scalar.activation(xn[s][ci][:, 1:1 + H, 1:1 + W],
                                         x_raw[s][ci][:], AF.Identity,
                                         bias=nmrs[:, 0:1], scale=rs[:, 0:1])

            def agg_weights(s):
                for ci in range(NCT):
                    triples = []
                    chunks = 3 if s == 0 else 1
                    for tr in range(chunks):
                        lo = tr * (9 // chunks) * O
                        hi = (tr + 1) * (9 // chunks) * O
                        ac = acc_pool.tile([128, hi - lo], F32, tag="acc")
                        nc.vector.tensor_scalar(
                            ac[:], wt_sb[0][ci][:, lo:hi],
                            attn_bc[s][:, 0:1], None, ALU.mult)
                        for k in (1, 2):
                            nc.vector.scalar_tensor_tensor(
                                ac[:], wt_sb[k][ci][:, lo:hi],
                                attn_bc[s][:, k:k + 1], ac[:],
                                ALU.mult, ALU.add)
                        aw = (aggw3_pool if chunks == 3 else aggw_pool).tile(
                            [128, (9 // chunks), O], BF16,
                            tag="aggw3" if chunks == 3 else "aggw")
                        nc.vector.scalar_tensor_tensor(
                            aw[:].rearrange("p a b -> p (a b)"),
                            wt_sb[3][ci][:, lo:hi],
                            attn_bc[s][:, 3:4], ac[:], ALU.mult, ALU.add)
                        triples.append(aw)
                    aggw[s][ci] = triples

            def warm_pe(stage):
                # keep the PE busy (HAM warm) while the aggregation finishes;
                # results go to a scratch psum bank and are never read.
                if stage == 0:
                    g = x_raw[0][0][:].rearrange("p a b -> p (a b)").bitcast(BF16)
                    lhsT, rhs = g[:, 0:128], g[:, 512:1024]
                else:
                    xf = xn[0][0][:].rearrange("p a b -> p (a b)")
                    lhsT, rhs = xf[:, 0:128], xf[:, 512:1024]
                wp = cpsum_pool.tile([128, 512], F32, tag="warm", bufs=1)
                for i in range(16):
                    nc.tensor.matmul(wp[:], lhsT, rhs, start=True, stop=True)

            def lhsT_for(s, ci, t, oi):
                triples = aggw[s][ci]
                if len(triples) == 3:
                    return triples[t // 3][:, t % 3, oi * 128:(oi + 1) * 128]
                return triples[0][:, t, oi * 128:(oi + 1) * 128]

            def conv_otile(s, oi):
                for q in range(4):
                    ps = cpsum_pool.tile([128, 1024], F32, tag="cps")
                    for ci in range(NCT):
                        for t, (dy, dx) in enumerate(TAPS):
                            lhsT = lhsT_for(s, ci, t, oi)
                            first = (ci == 0 and t == 0)
                            last = (ci == NCT - 1 and t == len(TAPS) - 1)
                            for blk in range(2):
                                y0 = q * 16 + blk * 8
                                rhs = xn[s][ci][:, y0 + 1 + dy:y0 + 9 + dy,
                                                1 + dx:1 + dx + W]
                                nc.tensor.matmul(
                                    ps[:, blk * 512:(blk + 1) * 512],
                                    lhsT, rhs, start=first, stop=last)
                    ot = out_pool.tile([128, 1024], F32, tag="ot")
                    nc.vector.tensor_scalar(ot[:], ps[:],
                                            aggb_sb[s][oi][:, 0:1], None,
                                            ALU.add)
                    nc.sync.dma_start(
                        out=outap[s, oi * 128:(oi + 1) * 128,
                                  q * 16:(q + 1) * 16, :],
                        in_=ot[:])

            # ---- emission schedule ----
            stats(0)
            attention_mlp(0)
            agg_bias(0)
            normalize(0)
            agg_weights(0)
            warm_pe(0)
            warm_pe(1)
            stats(1)
            conv_otile(0, 0)
            # rest of sample 1 prep overlaps conv(0,0); its PE matmuls sit
            # after conv(0,0) in the PE queue.
            attention_mlp(1)
            normalize(1)
            agg_weights(1)
            conv_otile(0, 1)
            agg_bias(1)
            conv_otile(1, 0)
            conv_otile(1, 1)

    nc.compile()
    return nc


_CACHED = {}


def _get_program():
    if "nc" not in _CACHED:
        _CACHED["nc"] = build_program()
    return _CACHED["nc"]


def _prep_shared(weight, bias, fc1_w, fc1_b, fc2_w, fc2_b):
    # weight [K, O, C, 3, 3] -> [K, C, 3*3, O] -> [K, NCT, 128, 9*O], bf16
    wt = np.ascontiguousarray(weight.transpose(0, 2, 3, 4, 1)).reshape(
        K, NCT, 128, 9 * O).astype(ml_dtypes.bfloat16)
    fc1wT = np.ascontiguousarray(fc1_w.T).reshape(NCT, 128, K).astype(
        np.float32)
    fc2wT = np.ascontiguousarray(fc2_w.T).astype(np.float32)
    return {
        "wt": wt,
        "bias": bias.astype(np.float32),
        "fc1wT": fc1wT,
        "fc1b": fc1_b.reshape(K, 1).astype(np.float32),
        "fc2wT": fc2wT,
        "fc2b": fc2_b.reshape(K, 1).astype(np.float32),
        "e5": np.concatenate([np.eye(K, dtype=np.float32),
                              np.ones((K, 1), np.float32)], axis=1),
    }


def run(x, weight, bias, fc1_w, fc1_b, fc2_w, fc2_b, trace=False,
        trace_kwargs=None):
    nc = _get_program()
    shared = _prep_shared(weight, bias, fc1_w, fc1_b, fc2_w, fc2_b)
    x = np.asarray(x, dtype=np.float32)
    in_maps = []
    for i in range(N_CORES):
        m = dict(shared)
        m["x"] = np.ascontiguousarray(x[i * S:(i + 1) * S])
        in_maps.append(m)
    res = run_bass_kernel_spmd(nc, in_maps, core_ids=list(range(N_CORES)),
                               trace=trace, **(trace_kwargs or {}))
    out = np.concatenate([res.results[i]["out"] for i in range(N_CORES)],
                         axis=0)
    return out, res


def kernel(x, weight, bias, fc1_w, fc1_b, fc2_w, fc2_b):
    out, _ = run(x, weight, bias, fc1_w, fc1_b, fc2_w, fc2_b)
    return out


# revision 13
# speedup vs baseline: 1.1935x; 1.0081x over previous
"""DyConv (dynamic convolution) Trainium2 kernel.

Problem: B=16, C=256, O=256, K=4 experts, 3x3 same-conv on 64x64, with
per-sample attention over experts + InstanceNorm2d(affine=False) input norm.

Strategy: data-parallel over batch across 8 cores (2 samples/core).
Per core:
  - x (fp32) loads on the SP HWDGE ring; expert weight bank (bf16,
    host-pretransposed to [K, ctile, 128c, 9*256o]) on the ACT ring;
    small weights on the gpsimd (SWDGE) ring.
  - instance-norm stats via two ACT passes with accum_out (sum x, sum x^2);
    the activations' main outputs are dumped into the padded-xn interior,
    which the later normalize pass overwrites.  Attention consumes sum(x)
    directly (fc1wT host-scaled by 1/HW), so the MLP never waits on DVE.
  - attention MLP in fp32 on PE (tiny matmuls), softmax via Exp on ACT +
    partition-sum matmul + DRAM round-trip broadcast (gpsimd ring).
  - per-sample weight aggregation on DVE (scalar_tensor_tensor FMA chain);
    sample0/ctile0 chunked in tap-triples so conv can start early.
  - conv: per (sample, otile, quarter of 16 rows) accumulate 2 ctile x 9 tap
    bf16 matmuls into a 2-bank PSUM tile; drain on DVE fused with the
    aggregated bias; store on the SP ring.
  - emission order defers sample1's PE ops behind sample0's convs so the
    PE never waits on sample1's prep.
"""

import sys

sys.path.insert(0, "/opt/trn_rl_repo")

import numpy as np
import ml_dtypes

import concourse.bacc as bacc
import concourse.tile as tile
from concourse import mybir
from concourse.bass_utils import run_bass_kernel_spmd

F32 = mybir.dt.float32
BF16 = mybir.dt.bfloat16
AF = mybir.ActivationFunctionType
ALU = mybir.AluOpType

N_CORES = 8
S = 2          # samples per core
C = 256        # in channels
O = 256        # out channels
K = 4          # experts
H = W = 64
HP = WP = 66   # padded spatial
NCT = 2        # C tiles of 128
NOT = 2        # O tiles of 128
EPS = 1e-5
INV_HW = 1.0 / (H * W)
TAPS = [(dy, dx) for dy in (-1, 0, 1) for dx in (-1, 0, 1)]


def build_program():
    nc = bacc.Bacc("TRN2", target_bir_lowering=False, debug=False,
                   num_devices=N_CORES)

    x_d = nc.dram_tensor("x", [S, C, H, W], F32, kind="ExternalInput")
    wt_d = nc.dram_tensor("wt", [K, NCT, 128, 9 * O], BF16, kind="ExternalInput")
    bias_d = nc.dram_tensor("bias", [K, O], F32, kind="ExternalInput")
    fc1wT_d = nc.dram_tensor("fc1wT", [NCT, 128, K], F32, kind="ExternalInput")
    fc1b_d = nc.dram_tensor("fc1b", [K, 1], F32, kind="ExternalInput")
    fc2wT_d = nc.dram_tensor("fc2wT", [K, K], F32, kind="ExternalInput")
    fc2b_d = nc.dram_tensor("fc2b", [K, 1], F32, kind="ExternalInput")
    e5_d = nc.dram_tensor("e5", [K, K + 1], F32, kind="ExternalInput")
    out_d = nc.dram_tensor("out", [S, O, H, W], F32, kind="ExternalOutput")

    xap = x_d.ap()
    outap = out_d.ap()

    with tile.TileContext(nc) as tc:
        with (
            tc.tile_pool(name="singles", bufs=1) as singles,
            tc.tile_pool(name="xraw", bufs=4) as xraw_pool,
            tc.tile_pool(name="xn", bufs=4) as xn_pool,
            tc.tile_pool(name="acc", bufs=2) as acc_pool,
            tc.tile_pool(name="aggw", bufs=2) as aggw_pool,
            tc.tile_pool(name="aggw3", bufs=6) as aggw3_pool,
            tc.tile_pool(name="stats", bufs=4) as stats_pool,
            tc.tile_pool(name="small", bufs=2) as small_pool,
            tc.tile_pool(name="outs", bufs=3) as out_pool,
            tc.tile_pool(name="cpsum", bufs=4, space="PSUM") as cpsum_pool,
        ):
            # ---- constants ----
            eps_sb = singles.tile([128, 1], F32, tag="eps")
            nc.vector.memset(eps_sb[:], EPS)
            e5_sb = singles.tile([K, K + 1], F32, tag="e5")
            nc.gpsimd.dma_start(out=e5_sb[:], in_=e5_d.ap())
            ones1_sb = singles.tile([1, 128], F32, tag="ones1")
            nc.vector.memset(ones1_sb[:], 1.0)

            # small weights on the gpsimd ring (keep the SP ring free for x)
            fc1wT_sb = []
            for ci in range(NCT):
                t = singles.tile([128, K], F32, tag=f"fc1wT{ci}")
                nc.gpsimd.dma_start(out=t[:], in_=fc1wT_d.ap()[ci])
                fc1wT_sb.append(t)
            fc2wT_sb = singles.tile([K, K], F32, tag="fc2wT")
            nc.gpsimd.dma_start(out=fc2wT_sb[:], in_=fc2wT_d.ap())
            fc1b_sb = singles.tile([K, 1], F32, tag="fc1b")
            nc.gpsimd.dma_start(out=fc1b_sb[:], in_=fc1b_d.ap())
            fc2b_sb = singles.tile([K, 1], F32, tag="fc2b")
            nc.gpsimd.dma_start(out=fc2b_sb[:], in_=fc2b_d.ap())
            bias_sb = singles.tile([K, O], F32, tag="biasK")
            nc.gpsimd.dma_start(out=bias_sb[:], in_=bias_d.ap())

            # ---- big loads: one HWDGE ring, priority order ----
            x_raw = [[None] * NCT for _ in range(S)]
            for ci in range(NCT):
                t = xraw_pool.tile([128, H, W], F32, tag="xraw")
                for hh in range(2):
                    nc.sync.dma_start(
                        out=t[:, hh * 32:(hh + 1) * 32, :],
                        in_=xap[0, ci * 128:(ci + 1) * 128,
                                hh * 32:(hh + 1) * 32, :])
                x_raw[0][ci] = t

            wt_sb = [[None] * NCT for _ in range(K)]
            for ci in range(NCT):
                for k in range(K):
                    t = singles.tile([128, 9 * O], BF16, tag=f"wt{k}_{ci}")
                    nc.sync.dma_start(out=t[:], in_=wt_d.ap()[k, ci])
                    wt_sb[k][ci] = t

            for ci in range(NCT):
                t = xraw_pool.tile([128, H, W], F32, tag="xraw")
                nc.sync.dma_start(out=t[:], in_=xap[1, ci * 128:(ci + 1) * 128, :, :])
                x_raw[1][ci] = t

            # ---- padded-xn border memsets (tiny, gpsimd) ----
            xn = [[None] * NCT for _ in range(S)]
            for s in range(S):
                for ci in range(NCT):
                    xt = xn_pool.tile([128, HP, WP], BF16, tag="xn")
                    nc.gpsimd.memset(xt[:, 0, :], 0.0)
                    nc.gpsimd.memset(xt[:, HP - 1, :], 0.0)
                    nc.gpsimd.memset(xt[:, 1:HP - 1, 0], 0.0)
                    nc.gpsimd.memset(xt[:, 1:HP - 1, WP - 1], 0.0)
                    xn[s][ci] = xt

            mv = [[None] * NCT for _ in range(S)]
            attn_t = [None] * S
            attn_bc = [None] * S
            aggb_sb = [[None] * NOT for _ in range(S)]
            aggw = [[None] * NCT for _ in range(S)]

            def stats(s):
                # instance-norm stats on DVE (idle early); mean feeds attention
                for ci in range(NCT):
                    st = stats_pool.tile([128, 8, 6], F32, tag="bnstats")
                    for j in range(8):
                        nc.vector.bn_stats(
                            out=st[:, j, :],
                            in_=x_raw[s][ci][:, 8 * j:8 * (j + 1), :]
                            .rearrange("p a b -> p (a b)"))
                    m = stats_pool.tile([128, 2], F32, tag="mv")
                    nc.vector.bn_aggr(out=m[:], in_=st[:])
                    mv[s][ci] = m

            def attention_mlp(s):
                ph = cpsum_pool.tile([K, 1], F32, tag="cps")
                for ci in range(NCT):
                    nc.tensor.matmul(ph[:], fc1wT_sb[ci][:], mv[s][ci][:, 0:1],
                                     start=(ci == 0), stop=(ci == NCT - 1))
                h_sb = small_pool.tile([K, 1], F32, tag="h")
                nc.scalar.activation(h_sb[:], ph[:], AF.Relu, bias=fc1b_sb[:])
                pl = cpsum_pool.tile([K, 1], F32, tag="cps")
                nc.tensor.matmul(pl[:], fc2wT_sb[:], h_sb[:],
                                 start=True, stop=True)
                exp_t = small_pool.tile([K, 1], F32, tag="expt")
                nc.scalar.activation(exp_t[:], pl[:], AF.Exp, bias=fc2b_sb[:])
                # on-chip softmax broadcast: exp_t.T @ [eye|ones] gives the
                # exp row + its sum on partition 0; recip the sum; then
                # ones-column matmul broadcasts [e0..e3, 1/s] to all partitions
                p5 = cpsum_pool.tile([1, K + 1], F32, tag="cps")
                nc.tensor.matmul(p5[:], exp_t[:], e5_sb[:],
                                 start=True, stop=True)
                row5 = small_pool.tile([1, K + 1], F32, tag="row5")
                nc.vector.tensor_copy(row5[0:1, 0:K], p5[0:1, 0:K])
                nc.vector.reciprocal(out=row5[0:1, K:K + 1],
                                     in_=p5[0:1, K:K + 1])
                pbc = cpsum_pool.tile([128, K + 1], F32, tag="cps")
                nc.tensor.matmul(pbc[:], ones1_sb[:], row5[:],
                                 start=True, stop=True)
                abc = small_pool.tile([128, K], F32, tag="attnbc")
                nc.vector.tensor_scalar(abc[:], pbc[:, 0:K],
                                        pbc[:, K:K + 1], None, ALU.mult)
                attn_bc[s] = abc
                at = small_pool.tile([K, 1], F32, tag="attnt")
                nc.vector.tensor_mul(at[:], exp_t[:], pbc[0:K, K:K + 1])
                attn_t[s] = at

            def agg_bias(s):
                for oi in range(NOT):
                    pab = cpsum_pool.tile([128, 1], F32, tag="cps")
                    nc.tensor.matmul(pab[:],
                                     bias_sb[:, oi * 128:(oi + 1) * 128],
                                     attn_t[s][:], start=True, stop=True)
                    ab = singles.tile([128, 1], F32, tag=f"aggb{s}_{oi}")
                    nc.vector.tensor_copy(ab[:], pab[:])
                    aggb_sb[s][oi] = ab

            def normalize(s):
                for ci in range(NCT):
                    # rsqrt(v+eps) = exp(-0.5*ln(v+eps)); ln and exp share one
                    # ACT table set, so the kernel never swaps tables
                    lv = stats_pool.tile([128, 1], F32, tag="lv")
                    nc.scalar.activation(lv[:], mv[s][ci][:, 1:2], AF.Ln,
                                         bias=eps_sb[:])
                    rs = stats_pool.tile([128, 1], F32, tag="rs")
                    nc.scalar.activation(rs[:], lv[:], AF.Exp, scale=-0.5)
                    nmrs = stats_pool.tile([128, 1], F32, tag="nmrs")
                    nc.vector.tensor_scalar(nmrs[:], mv[s][ci][:, 0:1],
                                            rs[:, 0:1], -1.0, ALU.mult,
                                            ALU.mult)
                    nc.scalar.activation(xn[s][ci][:, 1:1 + H, 1:1 + W],
                                         x_raw[s][ci][:], AF.Identity,
                                         bias=nmrs[:, 0:1], scale=rs[:, 0:1])

            def agg_weights(s):
                for ci in range(NCT):
                    triples = []
                    chunks = 3 if s == 0 else 1
                    for tr in range(chunks):
                        lo = tr * (9 // chunks) * O
                        hi = (tr + 1) * (9 // chunks) * O
                        ac = acc_pool.tile([128, hi - lo], F32, tag="acc")
                        nc.vector.tensor_scalar(
                            ac[:], wt_sb[0][ci][:, lo:hi],
                            attn_bc[s][:, 0:1], None, ALU.mult)
                        for k in (1, 2):
                            nc.vector.scalar_tensor_tensor(
                                ac[:], wt_sb[k][ci][:, lo:hi],
                                attn_bc[s][:, k:k + 1], ac[:],
                                ALU.mult, ALU.add)
                        aw = (aggw3_pool if chunks == 3 else aggw_pool).tile(
                            [128, (9 // chunks), O], BF16,
                            tag="aggw3" if chunks == 3 else "aggw")
                        nc.vector.scalar_tensor_tensor(
                            aw[:].rearrange("p a b -> p (a b)"),
                            wt_sb[3][ci][:, lo:hi],
                            attn_bc[s][:, 3:4], ac[:], ALU.mult, ALU.add)
                        triples.append(aw)
                    aggw[s][ci] = triples

            def warm_pe(stage):
                # keep the PE busy (HAM warm) while the aggregation finishes;
                # results go to a scratch psum bank and are never read.
                if stage == 0:
                    g = x_raw[0][0][:].rearrange("p a b -> p (a b)").bitcast(BF16)
                    lhsT, rhs = g[:, 0:128], g[:, 512:1024]
                else:
                    xf = xn[0][0][:].rearrange("p a b -> p (a b)")
                    lhsT, rhs = xf[:, 0:128], xf[:, 512:1024]
                wp = cpsum_pool.tile([128, 512], F32, tag="cps")
                for i in range(16):
                    nc.tensor.matmul(wp[:], lhsT, rhs, start=True, stop=True)

            def lhsT_for(s, ci, t, oi):
                triples = aggw[s][ci]
                if len(triples) == 3:
                    return triples[t // 3][:, t % 3, oi * 128:(oi + 1) * 128]
                return triples[0][:, t, oi * 128:(oi + 1) * 128]

            def conv_otile(s, oi):
                for q in range(4):
                    ps = cpsum_pool.tile([128, 1024], F32, tag="cps")
                    for ci in range(NCT):
                        for t, (dy, dx) in enumerate(TAPS):
                            lhsT = lhsT_for(s, ci, t, oi)
                            first = (ci == 0 and t == 0)
                            last = (ci == NCT - 1 and t == len(TAPS) - 1)
                            for blk in range(2):
                                y0 = q * 16 + blk * 8
                                rhs = xn[s][ci][:, y0 + 1 + dy:y0 + 9 + dy,
                                                1 + dx:1 + dx + W]
                                nc.tensor.matmul(
                                    ps[:, blk * 512:(blk + 1) * 512],
                                    lhsT, rhs, start=first, stop=last)
                    ot = out_pool.tile([128, 1024], F32, tag="ot")
                    nc.vector.tensor_scalar(ot[:], ps[:],
                                            aggb_sb[s][oi][:, 0:1], None,
                                            ALU.add)
                    nc.sync.dma_start(
                        out=outap[s, oi * 128:(oi + 1) * 128,
                                  q * 16:(q + 1) * 16, :],
                        in_=ot[:])

            # ---- emission schedule ----
            stats(0)
            attention_mlp(0)
            agg_bias(0)
            normalize(0)
            agg_weights(0)
            warm_pe(0)
            warm_pe(1)
            stats(1)
            conv_otile(0, 0)
            # rest of sample 1 prep overlaps conv(0,0); its PE matmuls sit
            # after conv(0,0) in the PE queue.
            attention_mlp(1)
            normalize(1)
            agg_weights(1)
            conv_otile(0, 1)
            agg_bias(1)
            conv_otile(1, 0)
            conv_otile(1, 1)

    nc.compile()
    return nc


_CACHED = {}


def _get_program():
    if "nc" not in _CACHED:
        _CACHED["nc"] = build_program()
    return _CACHED["nc"]


def _prep_shared(weight, bias, fc1_w, fc1_b, fc2_w, fc2_b):
    # weight [K, O, C, 3, 3] -> [K, C, 3*3, O] -> [K, NCT, 128, 9*O], bf16
    wt = np.ascontiguousarray(weight.transpose(0, 2, 3, 4, 1)).reshape(
        K, NCT, 128, 9 * O).astype(ml_dtypes.bfloat16)
    fc1wT = np.ascontiguousarray(fc1_w.T).reshape(NCT, 128, K).astype(
        np.float32)
    fc2wT = np.ascontiguousarray(fc2_w.T).astype(np.float32)
    return {
        "wt": wt,
        "bias": bias.astype(np.float32),
        "fc1wT": fc1wT,
        "fc1b": fc1_b.reshape(K, 1).astype(np.float32),
        "fc2wT": fc2wT,
        "fc2b": fc2_b.reshape(K, 1).astype(np.float32),
        "e5": np.concatenate([np.eye(K, dtype=np.float32),
                              np.ones((K, 1), np.float32)], axis=1),
    }


def run(x, weight, bias, fc1_w, fc1_b, fc2_w, fc2_b, trace=False,
        trace_kwargs=None):
    nc = _get_program()
    shared = _prep_shared(weight, bias, fc1_w, fc1_b, fc2_w, fc2_b)
    x = np.asarray(x, dtype=np.float32)
    in_maps = []
    for i in range(N_CORES):
        m = dict(shared)
        m["x"] = np.ascontiguousarray(x[i * S:(i + 1) * S])
        in_maps.append(m)
    res = run_bass_kernel_spmd(nc, in_maps, core_ids=list(range(N_CORES)),
                               trace=trace, **(trace_kwargs or {}))
    out = np.concatenate([res.results[i]["out"] for i in range(N_CORES)],
                         axis=0)
    return out, res


def kernel(x, weight, bias, fc1_w, fc1_b, fc2_w, fc2_b):
    out, _ = run(x, weight, bias, fc1_w, fc1_b, fc2_w, fc2_b)
    return out
